# revision 34
# baseline (speedup 1.0000x reference)
"""Trainium2 Bass kernel for a dense transformer block (MLA attention + SwiGLU MLP).

Problem: B=2, T=2048, D=2048, HQ=16, HKV=4, DH=128, RQ=512, RKV=256, DFF=8192.

Sharding: sequence-parallel over 8 cores — core c owns 512 query tokens
(batch c//4, block c%4). K/V projection for the core's batch is replicated
(cheap) so NO collectives are needed. Per-core inputs are prepared on the
host: token order is rotated so the core's queries are always tokens [0:512)
(identical SPMD program on every core), and the attention-mask columns are
rotated identically so softmax over keys is order-invariant.

Layout (everything transposed, [feature, token]):
  x is uploaded bf16; r1 = 1/(sqrt(mean x^2) + eps) from on-device squares.
  B1k = Wk1^T xT ; KT = Wk2^T B1k          ([512, 2048] bf16)
  B1v = Wv1^T xT ; Vn = (B1v^T Wv2) * r1   ([2048, 512] bf16, natural)
  A1 = (Wq1^T xT[:, :512]) * r1q ; QT = (Wq2/sqrt(DH))^T A1
  attention per head pair hp, with Wo fused in:
    L^T[k,q] = KT^T QT (psum) ; P = exp(L^T * r1k + biasmask) [* expm diag]
    acc += P (vector, bf16)   ; O^T += Vn^T P (psum accum)
    S = ones^T acc (2 matmuls); 1/S via a DRAM-bounce partition spread;
    Sinv broadcast via K=1 matmul into psum; OT = O^T * Sinv;
    x2acc[dm-pair] += Wo_pair^T OT  (psum drain via vector add)
  masking: fully-masked key tiles get bias=-50000 in the exp (per-core data);
  only the 4 diagonal tiles multiply by an explicit exp(mask) tensor (on
  gpsimd, consumed only at the section end so it never paces the loop).
  Scheduling: each head pair's normalize+Wo matmuls are deferred TWO head
  pairs and emitted one-per-kt as gap fillers inside the later pair's kt
  loop — the in-order tensor queue then never stalls on the 1/S round-trip,
  which keeps the PE in its high p-state (a stalled PE drops to half clock).
  The final two pairs' Wo interleaves with the rmsnorm2 squares.
  rmsnorm2: single-lane 1/(n+eps) (no DRAM bounce: those DMAs contend with
  the MLP weight stream); r2 broadcast via K=1 matmul; SwiGLU MLP as before.

norm1_w/norm2_w are folded into Wq1/Wk1/Wv1 and WupA/WupB on the host;
1/sqrt(DH) is folded into Wq2.
"""
import math
import numpy as np
import ml_dtypes

import concourse.bass as bass
import concourse.mybir as mybir
import concourse.tile as tile
from concourse import bacc
from concourse.bass_utils import run_bass_kernel_spmd
from contextlib import ExitStack

B, T, D = 2, 2048, 2048
HQ, HKV, DH = 16, 4, 128
RQ, RKV = 512, 256
DFF = 8192
EPS = 1e-5
NCORES = 8
Q = 512          # queries per core
P = 128
DT = D // P      # 16 d tiles
KT = T // P      # 16 key tiles
RQT = RQ // P    # 4
RKT = RKV // P   # 2
HQT = HQ         # 16 q-head tiles (DH=128)
HKVT = HKV       # 4 kv-head tiles
FT = DFF // P    # 64 dff tiles
GROUP = HQ // HKV
NDIAG = Q // P   # 4 diagonal key tiles

F32 = mybir.dt.float32
BF16 = mybir.dt.bfloat16
FP8 = mybir.dt.float8e4
BF = ml_dtypes.bfloat16
F8 = ml_dtypes.float8_e4m3

# fp8 power-of-2 scales: weights 2^SW (lifts sigma~0.02 out of subnormals),
# activations 2^SH / 2^SG keep magnitudes ~16-32 << 240 (TRN e4m3 max).
SW = 9
SH = 4
SG = 4
SV = 6        # V / normalized-O fp8 scale (attention values ~0.3 rms)
SX2 = 15      # residual stream carried as 2^15*x2 so fp8 Wo psum drains 1-op
SWI = False   # DoubleRowSwInterleave weight layout (contiguous ldweights)

_CACHE = {}


def _pm():
    return (mybir.MatmulPerfMode.DoubleRowSwInterleave if SWI
            else mybir.MatmulPerfMode.DoubleRow)


def _build_nc():
    nc = bacc.Bacc("TRN2", debug=False, num_devices=NCORES)
    ap = {}
    def din(name, shape, dt=BF16):
        ap[name] = nc.dram_tensor(name, list(shape), dt, kind="ExternalInput").ap()
    din("xT", [D, T], BF16)
    din("expm", [NDIAG * P, 2, Q], BF16)   # exp(mask)^T, diagonal key tiles only
    din("bm", [P, KT], F32)                # 0 visible / -50000 fully-masked per key tile
    din("q1p", [RQT, P, DT, P])
    din("q2p", [HQT, P, RQT, P])
    din("k1p", [RKT, P, DT, P])
    din("k2p", [HKVT, P, RKT, P])
    din("v1p", [RKT, P, DT, P])
    din("v2n", [RKV, HKV * DH])
    din("woh", [HQT // 2, P, DT, 2, P], FP8)   # per-head-pair Wo, 2^SW-scaled
    din("uap", [FT, P, DT, P])
    din("ubp", [FT, P, DT, P], FP8)        # sigmoid-damped path affords fp8
    din("wdp", [DT, P, FT, P])
    outT = nc.dram_tensor("outT", [D, Q], F32, kind="ExternalOutput").ap()

    AL = mybir.AluOpType
    AF = mybir.ActivationFunctionType

    with tile.TileContext(nc) as tc, ExitStack() as ctx:
        const = ctx.enter_context(tc.tile_pool(name="const", bufs=1))
        dram = ctx.enter_context(tc.tile_pool(name="drsc", bufs=1, space="DRAM"))

        ones = const.tile([P, 1], BF16)
        nc.vector.memset(ones, 1.0)
        onesc = const.tile([1, P], F32)
        nc.vector.memset(onesc, 1.0)
        onesb = const.tile([1, P], BF16)
        nc.vector.memset(onesb, 1.0)
        bm = const.tile([P, KT], F32)
        nc.scalar.dma_start(out=bm, in_=ap["bm"])

        x2pool = ctx.enter_context(tc.tile_pool(name="x2", bufs=1))
        h2pool = ctx.enter_context(tc.tile_pool(name="h2", bufs=1))

        phkv = ExitStack()
        kvq = phkv.enter_context(tc.tile_pool(name="kvq", bufs=1))
        phh = ExitStack()
        hpool = phh.enter_context(tc.tile_pool(name="h", bufs=1))
        psa_st = ExitStack()
        psA = psa_st.enter_context(tc.tile_pool(name="psA", bufs=4, space="PSUM"))

        # =============== Phase 1: load bf16 x; r1 stats ===============
        # rmsnorm scaling commutes through the linear projections: matmuls run
        # on raw bf16 x, r1 is applied per-partition on V / via Exp scale on K
        # / as a token-wide broadcast on the Q path.
        wp2 = ExitStack()
        wpool2 = wp2.enter_context(tc.tile_pool(name="w2", bufs=3))

        ph1 = ExitStack()
        st1 = ph1.enter_context(tc.tile_pool(name="st1", bufs=1))
        ssqp = ph1.enter_context(tc.tile_pool(name="ssqp", bufs=1, space="PSUM"))
        ph1sq = ExitStack()
        sqpool = ph1sq.enter_context(tc.tile_pool(name="sq", bufs=3))

        # x tiles stream first (they gate the first squares/matmuls); the
        # small-weight prefetches ride the idle gpsimd queue.
        hT = []
        for i in range(DT):
            xb = hpool.tile([P, T], BF16, name=f"xb{i}", tag=f"xb{i}")
            nc.sync.dma_start(out=xb, in_=ap["xT"][i * P:(i + 1) * P, :])
            hT.append(xb)
            if i == 3:
                w_k1, w_v1 = [], []
                for rt in range(RKT):
                    w = wpool2.tile([P, DT, P], BF16, name="wk1", tag="w16")
                    nc.gpsimd.dma_start(out=w, in_=ap["k1p"][rt])
                    w_k1.append(w)
                for rt in range(RKT):
                    w = wpool2.tile([P, DT, P], BF16, name="wv1", tag="w16")
                    nc.gpsimd.dma_start(out=w, in_=ap["v1p"][rt])
                    w_v1.append(w)

        ssq = [ssqp.tile([1, 512], F32, name=f"ssq{c}", tag=f"ssq{c}") for c in range(4)]
        for i in range(DT):
            sq = sqpool.tile([P, T], BF16, name="sq", tag="sq")
            nc.scalar.square(sq, hT[i])
            for c in range(4):
                nc.tensor.matmul(ssq[c], lhsT=ones, rhs=sq[:, c * 512:(c + 1) * 512],
                                 start=(i == 0), stop=(i == DT - 1))
        nrow = st1.tile([1, T], F32)
        for c in range(4):
            nc.scalar.activation(nrow[:, c * 512:(c + 1) * 512], ssq[c],
                                 AF.Sqrt, scale=1.0 / D)
        ph1sq.close()

        # =============== Phase 2: K/V/Q projections ===============
        # B1/KT matmuls do not need r1, so they are emitted first and the
        # r1 reciprocal chain (DRAM round-trip) hides beneath them.
        ph2 = ExitStack()
        bpool = ph2.enter_context(tc.tile_pool(name="b1", bufs=1))
        wq2pool = ph2.enter_context(tc.tile_pool(name="wq2p", bufs=3))

        B1 = {}
        for nm, wlist in (("k", w_k1), ("v", w_v1)):
            for rt in range(RKT):
                w = wlist[rt]
                bt = bpool.tile([P, T], BF16, name=f"B1{nm}{rt}", tag=f"B1{nm}{rt}")
                for c in range(4):
                    pst = psA.tile([P, 512], F32, name="ps", tag="ps")
                    for i in range(DT):
                        nc.tensor.matmul(pst, lhsT=w[:, i, :],
                                         rhs=hT[i][:, c * 512:(c + 1) * 512],
                                         start=(i == 0), stop=(i == DT - 1))
                    if c % 2 == 0:
                        nc.scalar.activation(bt[:, c * 512:(c + 1) * 512], pst,
                                             AF.Copy)
                    else:
                        nc.vector.tensor_copy(out=bt[:, c * 512:(c + 1) * 512],
                                              in_=pst)
                B1[(nm, rt)] = bt

        v2sb = kvq.tile([P, RKT, HKV * DH], BF16, name="v2", tag="v2")
        nc.scalar.dma_start(out=v2sb, in_=ap["v2n"].rearrange("(kt p) n -> p kt n", p=P))

        KTs = []
        for hd in range(HKVT):
            w = wq2pool.tile([P, RKT, P], BF16, name="wk2", tag="wk2")
            nc.scalar.dma_start(out=w, in_=ap["k2p"][hd])
            kt_sb = kvq.tile([P, T], BF16, name=f"KT{hd}", tag=f"KT{hd}")
            for c in range(4):
                pst = psA.tile([P, 512], F32, name="ps", tag="ps")
                for rt in range(RKT):
                    nc.tensor.matmul(pst, lhsT=w[:, rt, :],
                                     rhs=B1[("k", rt)][:, c * 512:(c + 1) * 512],
                                     start=(rt == 0), stop=(rt == RKT - 1))
                if c % 2 == 0:
                    nc.scalar.activation(kt_sb[:, c * 512:(c + 1) * 512], pst,
                                         AF.Copy)
                else:
                    nc.vector.tensor_copy(out=kt_sb[:, c * 512:(c + 1) * 512],
                                          in_=pst)
            KTs.append(kt_sb)
            if hd == 0:
                # r1 chain: reciprocal on 128 partitions via DRAM bounce; keep
                # r1p [P, T//P] (token t at [t % P, t // P]). Emitted here so
                # the PE chews KT matmuls while the bounce is in flight.
                nd = dram.tile([1, T], F32, name="r1nd", tag="r1nd")
                nc.gpsimd.dma_start(out=nd, in_=nrow)
                np_sb = st1.tile([P, KT], F32, name="np_sb", tag="np_sb")
                nc.gpsimd.dma_start(out=np_sb, in_=nd[0].rearrange("(t p) -> p t", p=P))
                nc.vector.tensor_scalar_add(np_sb, np_sb, EPS)
                r1p = const.tile([P, KT], F32)
                nc.vector.reciprocal(r1p, np_sb)
                r1row = st1.tile([1, Q], F32)
                nc.vector.tensor_scalar_add(r1row, nrow[:, 0:Q], EPS)
                nc.vector.reciprocal(r1row, r1row)

        # r512: per-query 1/(n+eps) broadcast across partitions via K=1 matmul
        rps = psA.tile([P, 512], F32, name="ps", tag="ps")
        nc.tensor.matmul(rps, lhsT=onesc, rhs=r1row, start=True, stop=True)
        r512 = const.tile([P, Q], F32)
        nc.vector.tensor_copy(out=r512, in_=rps)

        # V in fp8 (scale 2^SV), keys on partitions, kt-major so DoubleRow PV
        # can take kt-pairs; r1 (key rmsnorm) folded into the scale.
        r1pv = const.tile([P, KT], F32)
        nc.vector.tensor_scalar_mul(r1pv, r1p, float(2.0 ** SV))
        vq = kvq.tile([P, KT, HKV * DH], FP8, name="vq", tag="vq")
        for t in range(KT):
            pst = psA.tile([P, 512], F32, name="ps", tag="ps")
            for rt in range(RKT):
                nc.tensor.matmul(pst, lhsT=B1[("v", rt)][:, t * P:(t + 1) * P],
                                 rhs=v2sb[:, rt, :],
                                 start=(rt == 0), stop=(rt == RKT - 1))
            nc.vector.tensor_scalar_mul(vq[:, t, :], pst, r1pv[:, t:t + 1])

        A1 = []
        for rt in range(RQT):
            w = wpool2.tile([P, DT, P], BF16, name="wq1", tag="w16")
            nc.scalar.dma_start(out=w, in_=ap["q1p"][rt])
            pst = psA.tile([P, 512], F32, name="ps", tag="ps")
            for i in range(DT):
                nc.tensor.matmul(pst, lhsT=w[:, i, :], rhs=hT[i][:, 0:Q],
                                 start=(i == 0), stop=(i == DT - 1))
            a = bpool.tile([P, Q], BF16, name=f"A1{rt}", tag=f"A1{rt}")
            nc.vector.tensor_tensor(a, pst, r512, AL.mult)
            A1.append(a)
        QTs = []
        for hd in range(HQT):
            w = wq2pool.tile([P, RQT, P], BF16, name="wq2", tag="wq2")
            nc.scalar.dma_start(out=w, in_=ap["q2p"][hd])
            pst = psA.tile([P, 512], F32, name="ps", tag="ps")
            for rt in range(RQT):
                nc.tensor.matmul(pst, lhsT=w[:, rt, :], rhs=A1[rt],
                                 start=(rt == 0), stop=(rt == RQT - 1))
            qt = kvq.tile([P, Q], BF16, name=f"QT{hd}", tag=f"QT{hd}")
            if hd % 2 == 0:
                nc.scalar.activation(qt, pst, AF.Copy)
            else:
                nc.vector.tensor_copy(out=qt, in_=pst)
            QTs.append(qt)

        # residual accumulator (bf16), seeded with x; Wo partial sums add in
        # during attention, and the MLP reads/adds it at the end.
        # residual accumulator carried as 2^SX2 * x2 (bf16 exponent absorbs
        # it) so the 2^(SW+SV)-scaled fp8 Wo psum adds in with no extra op
        x2a = x2pool.tile([P, DT, Q], BF16, name="x2a", tag="x2a")
        for dm in range(DT):
            nc.vector.tensor_scalar_mul(x2a[:, dm, :], hT[dm][:, 0:Q],
                                        float(2.0 ** SX2))

        ph2.close()
        ph1.close()
        wp2.close()
        psa_st.close()
        phh.close()

        # =============== Phase 3: attention + Wo, fused ===============
        ph3 = ExitStack()
        mpool = ph3.enter_context(tc.tile_pool(name="mask", bufs=1))
        ptp = ph3.enter_context(tc.tile_pool(name="ptp", bufs=4))
        ptdp = ph3.enter_context(tc.tile_pool(name="ptdp", bufs=8))
        accp = ph3.enter_context(tc.tile_pool(name="accp", bufs=2))
        otp = ph3.enter_context(tc.tile_pool(name="otp", bufs=3))
        svp = ph3.enter_context(tc.tile_pool(name="svp", bufs=3))
        wohp = ph3.enter_context(tc.tile_pool(name="wohp", bufs=6))
        wops = ph3.enter_context(tc.tile_pool(name="wops", bufs=1, space="PSUM"))
        ph3a = ExitStack()
        plp = ph3a.enter_context(tc.tile_pool(name="plp", bufs=2, space="PSUM"))
        pop = ph3a.enter_context(tc.tile_pool(name="pop", bufs=1, space="PSUM"))

        expm_sb = []
        for kt in range(NDIAG):
            et = mpool.tile([P, 2, Q], BF16, name=f"em{kt}", tag=f"em{kt}")
            nc.scalar.dma_start(out=et, in_=ap["expm"][kt * P:(kt + 1) * P])
            expm_sb.append(et)

        # Software-pipelined emission: head-pair hp's normalize+Wo matmuls are
        # emitted one-by-one as gap FILLERS inside hp+1's kt loop, so the
        # in-order tensor queue never stalls (a stalled PE drops out of its
        # high p-state and runs matmuls at half clock). PV for kt is emitted
        # one iteration late so exp(kt) is always done when the queue reaches
        # it. Diagonal (partially-masked) key tiles are emitted FIRST (their
        # exp output needs a slow gpsimd expm-multiply) with their acc/PV
        # consumers LAST.
        nondiag = list(range(NDIAG, KT))

        DR = mybir.MatmulPerfMode.DoubleRow

        def wo_fillers(sts):
            """Per-tensor-queue-slot closures for 1-2 head pairs' normalize +
            Wo. Both pairs' contributions accumulate in the same psum tile so
            each x2 drain covers two head pairs."""
            for st in sts:
                def bcast(st=st):
                    wps2 = wops.tile([P, 2, Q], F32, name="wo", tag="wo")
                    for z in (0, 1):
                        nc.tensor.matmul(wps2[:, z, :], lhsT=onesb,
                                         rhs=st["sinv"][0:1, z * Q:(z + 1) * Q],
                                         start=True, stop=True)
                    # otn = 2^SV * O/S in fp8 (po carries 2^(SV+SP), sinv 2^-SP)
                    otn = otp.tile([P, 2, Q], FP8, name="otn", tag="otn")
                    nc.vector.tensor_tensor(otn, wps2, st["otr"], AL.mult)
                    st["otn"] = otn
                yield bcast
            for dp in range(DT // 2):
                def pair(dp=dp):
                    wps3 = wops.tile([P, 2, Q], F32, name="wo", tag="wo")
                    n = len(sts)
                    for si, st in enumerate(sts):
                        otn = st["otn"]
                        for z in (0, 1):
                            dm = 2 * dp + z
                            nc.tensor.matmul(wps3[:, z, :],
                                             lhsT=st["wh"][:, dm, :, :],
                                             rhs=otn,
                                             start=(si == 0), stop=(si == n - 1),
                                             perf_mode=DR)
                    nc.vector.tensor_tensor(x2a[:, 2 * dp:2 * dp + 2, :],
                                            x2a[:, 2 * dp:2 * dp + 2, :], wps3,
                                            AL.add)
                yield pair

        def s_block(st):
            """Deferred S + 1/S chain for a finished head pair: the ones^T acc
            reduction matmuls and the DRAM partition-spread reciprocal.
            Emitted as a filler inside the NEXT pair's kt loop so the in-order
            PE queue never waits on the DVE acc chain."""
            def emit(st=st):
                acc = st.pop("acc")
                spl = wops.tile([P, 2, Q], F32, name="wo", tag="wo")
                for z in (0, 1):
                    nc.tensor.matmul(spl[0:1, z, :], lhsT=ones, rhs=acc[:, z, :],
                                     start=True, stop=True)
                s_sb = svp.tile([1, 2 * Q], F32, name="ssb", tag="ssb")
                nc.scalar.activation(s_sb,
                                     spl[0:1, :, :].rearrange("o a q -> o (a q)"),
                                     AF.Copy)
                sd = dram.tile([1, 2 * Q], F32, name="sd", tag="sd")
                nc.sync.dma_start(out=sd, in_=s_sb)
                sp = svp.tile([P, 2 * Q // P], F32, name="sp", tag="sp")
                nc.sync.dma_start(out=sp, in_=sd[0].rearrange("(c p) -> p c", p=P))
                rp = svp.tile([P, 2 * Q // P], F32, name="rp", tag="rp")
                nc.vector.reciprocal(rp, sp)
                rpb = svp.tile([P, 2 * Q // P], BF16, name="rpb", tag="rpb")
                nc.vector.tensor_copy(out=rpb, in_=rp)
                rd = dram.tile([2 * Q // P, P], BF16, name="rd", tag="rd")
                nc.sync.dma_start(out=rd.rearrange("c p -> p c"), in_=rpb)
                sinv_row = svp.tile([1, 2 * Q], BF16, name="sinvr", tag="sinvr")
                nc.sync.dma_start(out=sinv_row, in_=rd.rearrange("c p -> (c p)"))
                st["sinv"] = sinv_row
            yield emit

        import itertools as _it
        tails = []
        fill = iter(())
        for hp in range(HQ // 2):
            h0, h1 = 2 * hp, 2 * hp + 1
            hk = h0 // GROUP
            wh = wohp.tile([P, DT, 2, P], FP8, name="wh", tag="wh")
            nc.sync.dma_start(out=wh, in_=ap["woh"][hp])

            gens = []
            if tails:
                gens.append(s_block(tails[-1]))
            if len(tails) >= 2:
                gens.append(wo_fillers([tails.pop(0)]))
            fill = _it.chain(*gens)

            po = pop.tile([P, 2, Q], F32, name="po", tag="po")
            acc = accp.tile([P, 2, Q], BF16, name="acca", tag="acca")

            def qk(kt, tag, bufs_pool):
                pl = plp.tile([P, 2, Q], F32, name="plp", tag="plp")
                for z in (0, 1):
                    nc.tensor.matmul(pl[:, z, :],
                                     lhsT=KTs[hk][:, kt * P:(kt + 1) * P],
                                     rhs=QTs[(h0, h1)[z]], start=True, stop=True)
                pt = bufs_pool.tile([P, 2, Q], BF16, name="pt", tag=tag)
                nc.scalar.activation(pt, pl, AF.Exp, scale=r1p[:, kt:kt + 1],
                                     bias=bm[:, kt:kt + 1])
                return pt

            def filler():
                f = next(fill, None)
                if f is not None:
                    f()

            ptd = []
            for kt in range(NDIAG):
                pt = qk(kt, "ptd", ptdp)
                filler()
                nc.gpsimd.tensor_tensor(pt, pt, expm_sb[kt], AL.mult)
                ptd.append(pt)
            # nondiag key tiles in PAIRS: exp straight to fp8, PV as one
            # DoubleRow matmul per (pair, head) contracting 256 keys. PV for
            # pair i is emitted during pair i+1 so exp is always done.
            NPAIR = (KT - NDIAG) // 2
            prev = None
            for ipr in range(NPAIR):
                kt0 = NDIAG + 2 * ipr
                ptt = ptp.tile([P, 2, 2, Q], FP8, name="pt", tag="pt")
                for j in (0, 1):
                    kt = kt0 + j
                    pl = plp.tile([P, 2, Q], F32, name="plp", tag="plp")
                    for z in (0, 1):
                        nc.tensor.matmul(pl[:, z, :],
                                         lhsT=KTs[hk][:, kt * P:(kt + 1) * P],
                                         rhs=QTs[(h0, h1)[z]],
                                         start=True, stop=True)
                    nc.scalar.activation(ptt[:, j, :, :], pl, AF.Exp,
                                         scale=r1p[:, kt:kt + 1],
                                         bias=bm[:, kt:kt + 1])
                    filler()
                    if ipr == 0 and j == 0:
                        nc.vector.tensor_copy(out=acc, in_=ptt[:, 0, :, :])
                    else:
                        nc.vector.tensor_tensor(acc, acc, ptt[:, j, :, :],
                                                AL.add)
                if prev is not None:
                    pptt, pkt0 = prev
                    for z in (0, 1):
                        nc.tensor.matmul(po[:, z, :],
                                         lhsT=vq[:, pkt0:pkt0 + 2,
                                                 hk * DH:(hk + 1) * DH],
                                         rhs=pptt[:, :, z, :],
                                         start=(ipr == 1), stop=False,
                                         perf_mode=DR)
                prev = (ptt, kt0)
            pptt, pkt0 = prev
            for z in (0, 1):
                nc.tensor.matmul(po[:, z, :],
                                 lhsT=vq[:, pkt0:pkt0 + 2, hk * DH:(hk + 1) * DH],
                                 rhs=pptt[:, :, z, :], start=False, stop=False,
                                 perf_mode=DR)
            for kt in range(NDIAG):
                pt = ptd[kt]
                nc.vector.tensor_tensor(acc, acc, pt, AL.add)
                for z in (0, 1):
                    nc.tensor.matmul(po[:, z, :],
                                     lhsT=vq[:, kt, hk * DH:(hk + 1) * DH],
                                     rhs=pt[:, z, :],
                                     start=False, stop=(kt == NDIAG - 1))
                filler()
            # po -> bf16 on the vector engine (scalar is pacing the exps);
            # S + 1/S is deferred into the next pair's filler stream.
            otr = otp.tile([P, 2, Q], BF16, name="otr", tag="otr")
            nc.vector.tensor_copy(out=otr, in_=po)
            for f in fill:
                f()
            tails.append({"acc": acc, "otr": otr, "wh": wh})
        # =============== Phase 4: rmsnorm2 (inside attention scope so the
        # final head pairs' Wo work interleaves with the squares; psum comes
        # from the wops pool since all 8 banks are still reserved) ===========
        ph3a.close()
        ph4 = ExitStack()
        st2 = ph4.enter_context(tc.tile_pool(name="st2", bufs=1))
        sq2pool = ph4.enter_context(tc.tile_pool(name="sq2", bufs=3))
        ssq2p = ph4.enter_context(tc.tile_pool(name="ssq2p", bufs=1, space="PSUM"))
        r2ps_p = ph4.enter_context(tc.tile_pool(name="r2ps", bufs=1, space="PSUM"))

        next(s_block(tails[1]))()    # S chain for the last head pair
        fin0 = wo_fillers([tails[0]])
        fin1 = wo_fillers([tails[1]])
        for f in fin0:
            f()
        next(fin1)()   # bcast for the last head pair
        ssq2 = ssq2p.tile([1, Q], F32, name="ssq2", tag="ssq2")
        for dm in range(DT):
            if dm % 2 == 0:
                next(fin1, lambda: None)()
            sq2 = sq2pool.tile([P, Q], BF16, name="sq2", tag="sq2")
            nc.scalar.square(sq2, x2a[:, dm, :])
            nc.tensor.matmul(ssq2, lhsT=ones, rhs=sq2,
                             start=(dm == 0), stop=(dm == DT - 1))
        n2 = st2.tile([1, Q], F32)
        nc.scalar.activation(n2, ssq2, AF.Sqrt, scale=1.0 / D)
        # single-lane 1/(n2+eps): slower per element than a partition-spread,
        # but avoids DRAM round-trips that contend with the MLP weight stream.
        # n2 is 2^SX2-scaled (x2a is), so eps scales too and r2 carries
        # 2^-SX2, cancelling the residual scale in h2 = r2*x2a.
        nc.vector.tensor_scalar_add(n2, n2, float(EPS * 2.0 ** SX2))
        r2f = st2.tile([1, Q], F32, name="r2f", tag="r2f")
        nc.vector.reciprocal(r2f, n2)
        r2row = st2.tile([1, Q], BF16, name="r2row", tag="r2row")
        nc.vector.tensor_copy(out=r2row, in_=r2f)
        r2ps = r2ps_p.tile([P, Q], F32, name="r2b", tag="r2b")
        nc.tensor.matmul(r2ps, lhsT=onesb, rhs=r2row, start=True, stop=True)
        r2b = st2.tile([P, Q], BF16, name="r2bs", tag="r2bs")
        nc.scalar.activation(r2b, r2ps, AF.Copy)
        # second broadcast scaled 2^SH: h2q = 2^SH * h2 in fp8 (up-B path);
        # produced on the idle gpsimd engine to spare DVE at the boundary
        r2b16 = st2.tile([P, Q], BF16, name="r2b16", tag="r2b16")
        nc.scalar.activation(r2b16, r2ps, AF.Copy, scale=float(2.0 ** SH))
        h2b = h2pool.tile([P, DT, Q], BF16, name="h2b", tag="h2b")
        h2q = h2pool.tile([P, DT, Q], FP8, name="h2q", tag="h2q")
        for dm in range(DT):
            nc.vector.tensor_tensor(h2b[:, dm, :], r2b, x2a[:, dm, :], AL.mult)
            nc.gpsimd.tensor_tensor(h2q[:, dm, :], r2b16, x2a[:, dm, :], AL.mult)
        ph4.close()
        ph3.close()
        phkv.close()

        # =============== Phase 5: SwiGLU MLP + residual ===============
        ph5 = ExitStack()
        gpool = ph5.enter_context(tc.tile_pool(name="g", bufs=1))
        psW = ph5.enter_context(tc.tile_pool(name="psW", bufs=4, space="PSUM"))
        psb = ph5.enter_context(tc.tile_pool(name="psb", bufs=4, space="PSUM"))
        wpool = ph5.enter_context(tc.tile_pool(name="w5", bufs=6))
        spool = ph5.enter_context(tc.tile_pool(name="sig", bufs=3))
        wdpool = ph5.enter_context(tc.tile_pool(name="wd", bufs=4))
        opool = ph5.enter_context(tc.tile_pool(name="out", bufs=3))

        PM = _pm()
        DP = DT // 2        # 8 contraction k-pairs for the fp8 up-B path
        g = []
        for f in range(FT):
            wa = wpool.tile([P, DT, P], BF16, name="w16", tag="w16")
            nc.sync.dma_start(out=wa, in_=ap["uap"][f])
            wb = wpool.tile([P, DT, P], FP8, name="w16b", tag="w16b")
            nc.sync.dma_start(out=wb, in_=ap["ubp"][f])
            pa = psW.tile([P, 512], F32, name="ps", tag="ps")
            pb = psb.tile([P, 512], F32, name="psb", tag="psb")
            for i in range(DT):
                nc.tensor.matmul(pa, lhsT=wa[:, i, :], rhs=h2b[:, i, :],
                                 start=(i == 0), stop=(i == DT - 1))
            for i in range(DP):
                nc.tensor.matmul(pb, lhsT=wb[:, 2 * i:2 * i + 2, :],
                                 rhs=h2q[:, 2 * i:2 * i + 2, :],
                                 start=(i == 0), stop=(i == DP - 1),
                                 perf_mode=PM)
            # pb holds 2^(SW+SH)*b
            sig = spool.tile([P, Q], F32, name="sig", tag="sig")
            nc.scalar.activation(sig, pb, AF.Sigmoid,
                                 scale=float(2.0 ** -(SW + SH)))
            gt = gpool.tile([P, Q], BF16, name=f"g{f}", tag=f"g{f}")
            nc.vector.tensor_tensor(gt, pa, sig, AL.mult)
            g.append(gt)

        H = FT // 2
        for dm in range(DT):
            wd0 = wdpool.tile([P, H, P], BF16, name="wd", tag="wd")
            nc.sync.dma_start(out=wd0, in_=ap["wdp"][dm, :, 0:H, :])
            wd1 = wdpool.tile([P, H, P], BF16, name="wd", tag="wd")
            nc.sync.dma_start(out=wd1, in_=ap["wdp"][dm, :, H:FT, :])
            pst = psW.tile([P, 512], F32, name="ps", tag="ps")
            for f in range(FT):
                wd = wd0 if f < H else wd1
                nc.tensor.matmul(pst, lhsT=wd[:, f % H, :], rhs=g[f],
                                 start=(f == 0), stop=(f == FT - 1))
            xsc = opool.tile([P, Q], BF16, name="xsc", tag="xsc")
            nc.scalar.activation(xsc, x2a[:, dm, :], AF.Copy,
                                 scale=float(2.0 ** -SX2))
            ot = opool.tile([P, Q], F32, name="outt", tag="outt")
            nc.vector.tensor_tensor(ot, pst, xsc, AL.add)
            nc.sync.dma_start(out=outT[dm * P:(dm + 1) * P, :], in_=ot)
        ph5.close()

    nc.compile()
    return nc


def _pack_lhsT(w):
    """[K, M] -> [M/128, 128, K/128, 128] so that out[mt, p, kt, c] = w[kt*128+p, mt*128+c]."""
    K, M = w.shape
    kt, mt = K // P, M // P
    return np.ascontiguousarray(
        w.reshape(kt, P, mt, P).transpose(2, 1, 0, 3)).astype(BF)


def _pack8(w, lg2s=SW):
    """fp8 DoubleRow pack: like _pack_lhsT but scaled 2^lg2s and cast e4m3.
    With SWI, each kt-pair block is stored column-interleaved+reversed
    ([A127,B127,...,A0,B0]) as DoubleRowSwInterleave expects."""
    K, M = w.shape
    kt, mt = K // P, M // P
    v = np.clip(np.asarray(w, np.float32) * (2.0 ** lg2s), -240, 240)
    q = np.ascontiguousarray(v.reshape(kt, P, mt, P).transpose(2, 1, 0, 3))
    if SWI:
        r = q.reshape(mt, P, kt // 2, 2, P)[..., ::-1]
        q = np.ascontiguousarray(r.transpose(0, 1, 2, 4, 3)).reshape(
            mt, P, kt, P)
    return q.astype(F8)


def prepare_in_maps(inputs):
    """Build the 8 per-core input dicts from the full-problem input arrays."""
    x = np.asarray(inputs["x"], np.float32)
    mask = np.asarray(inputs["attn_mask"], np.float32)[0, 0]   # [T, T]
    w1 = np.asarray(inputs["norm1_w"], np.float32)[:, None]
    w2 = np.asarray(inputs["norm2_w"], np.float32)[:, None]

    shared = {
        "q1p": _pack_lhsT(w1 * np.asarray(inputs["Wq1"], np.float32)),
        "q2p": _pack_lhsT(np.asarray(inputs["Wq2"], np.float32) / math.sqrt(DH)),
        "k1p": _pack_lhsT(w1 * np.asarray(inputs["Wk1"], np.float32)),
        "k2p": _pack_lhsT(np.asarray(inputs["Wk2"], np.float32)),
        "v1p": _pack_lhsT(w1 * np.asarray(inputs["Wv1"], np.float32)),
        "v2n": np.asarray(inputs["Wv2"], np.float32).astype(BF),
        "woh": np.ascontiguousarray(
            (np.asarray(inputs["Wo"], np.float32) * (2.0 ** SW))
            .reshape(HQT // 2, 2, P, DT, P).transpose(0, 2, 3, 1, 4)).astype(F8),
        "uap": _pack_lhsT(w2 * np.asarray(inputs["W_upA"], np.float32)),
        "ubp": _pack8(w2 * np.asarray(inputs["W_upB"], np.float32)),
        "wdp": _pack_lhsT(np.asarray(inputs["W_down"], np.float32)),
    }

    in_maps = []
    for c in range(NCORES):
        b, j = c // 4, c % 4
        xp = np.roll(x[b], -Q * j, axis=0)                     # [T, D]
        xbT = np.ascontiguousarray(xp.T).astype(BF)            # [D, T] bf16
        mq = np.roll(mask[Q * j:Q * (j + 1), :], -Q * j, axis=1)   # [Q, T]
        e1 = np.exp(mq[:, 0:Q]).T.astype(np.float32)           # [Q(keys), Q] diag block
        expm = np.ascontiguousarray(
            np.stack([e1, e1], axis=1)).astype(BF)             # [512, 2, 512]
        # key tile kt holds tokens (kt*128 + 512j) mod 2048: tiles >= 16-4j
        # wrapped to past tokens (visible); 4 <= kt < 16-4j are future (masked).
        # -ln2 halves every exp output (fp8 headroom); S sums the same halved
        # values so the normalization cancels the factor exactly.
        bm = np.full((P, KT), -math.log(2.0), np.float32)
        for kt in range(NDIAG, KT):
            if kt < KT - 4 * j:
                bm[:, kt] = -50000.0
        m = dict(shared)
        m["xT"] = xbT
        m["expm"] = expm
        m["bm"] = bm
        in_maps.append(m)
    return in_maps


def kernel(x, attn_mask, norm1_w, norm2_w, Wq1, Wq2, Wk1, Wk2, Wv1, Wv2, Wo,
           W_upA, W_upB, W_down):
    if "nc" not in _CACHE:
        _CACHE["nc"] = _build_nc()
    nc = _CACHE["nc"]

    in_maps = prepare_in_maps(dict(
        x=x, attn_mask=attn_mask, norm1_w=norm1_w, norm2_w=norm2_w,
        Wq1=Wq1, Wq2=Wq2, Wk1=Wk1, Wk2=Wk2, Wv1=Wv1, Wv2=Wv2, Wo=Wo,
        W_upA=W_upA, W_upB=W_upB, W_down=W_down))

    res = run_bass_kernel_spmd(nc, in_maps, core_ids=list(range(NCORES)))
    _CACHE["last_result"] = res

    out = np.empty((B, T, D), np.float32)
    for c in range(NCORES):
        b, j = c // 4, c % 4
        out[b, Q * j:Q * (j + 1), :] = res.results[c]["outT"].T
    return out



# revision 41
# speedup vs baseline: 1.0263x; 1.0263x over previous
"""Trainium2 Bass kernel for a dense transformer block (MLA attention + SwiGLU MLP).

Problem: B=2, T=2048, D=2048, HQ=16, HKV=4, DH=128, RQ=512, RKV=256, DFF=8192.

Sharding: sequence-parallel over 8 cores — core c owns 512 query tokens
(batch c//4, block c%4). K/V projection for the core's batch is replicated
(cheap) so NO collectives are needed. Per-core inputs are prepared on the
host: token order is rotated so the core's queries are always tokens [0:512)
(identical SPMD program on every core), and the attention-mask columns are
rotated identically so softmax over keys is order-invariant.

Layout (everything transposed, [feature, token]):
  x is uploaded bf16; r1 = 1/(sqrt(mean x^2) + eps) from on-device squares.
  B1k = Wk1^T xT ; KT = Wk2^T B1k          ([512, 2048] bf16)
  B1v = Wv1^T xT ; Vn = (B1v^T Wv2) * r1   ([2048, 512] bf16, natural)
  A1 = (Wq1^T xT[:, :512]) * r1q ; QT = (Wq2/sqrt(DH))^T A1
  attention per head pair hp, with Wo fused in:
    L^T[k,q] = KT^T QT (psum) ; P = exp(L^T * r1k + biasmask) [* expm diag]
    acc += P (vector, bf16)   ; O^T += Vn^T P (psum accum)
    S = ones^T acc (2 matmuls); 1/S via a DRAM-bounce partition spread;
    Sinv broadcast via K=1 matmul into psum; OT = O^T * Sinv;
    x2acc[dm-pair] += Wo_pair^T OT  (psum drain via vector add)
  masking: fully-masked key tiles get bias=-50000 in the exp (per-core data);
  only the 4 diagonal tiles multiply by an explicit exp(mask) tensor (on
  gpsimd, consumed only at the section end so it never paces the loop).
  Scheduling: each head pair's normalize+Wo matmuls are deferred TWO head
  pairs and emitted one-per-kt as gap fillers inside the later pair's kt
  loop — the in-order tensor queue then never stalls on the 1/S round-trip,
  which keeps the PE in its high p-state (a stalled PE drops to half clock).
  The final two pairs' Wo interleaves with the rmsnorm2 squares.
  rmsnorm2: single-lane 1/(n+eps) (no DRAM bounce: those DMAs contend with
  the MLP weight stream); r2 broadcast via K=1 matmul; SwiGLU MLP as before.

norm1_w/norm2_w are folded into Wq1/Wk1/Wv1 and WupA/WupB on the host;
1/sqrt(DH) is folded into Wq2.
"""
import math
import numpy as np
import ml_dtypes

import concourse.bass as bass
import concourse.mybir as mybir
import concourse.tile as tile
from concourse import bacc
from concourse.bass_utils import run_bass_kernel_spmd
from contextlib import ExitStack

B, T, D = 2, 2048, 2048
HQ, HKV, DH = 16, 4, 128
RQ, RKV = 512, 256
DFF = 8192
EPS = 1e-5
NCORES = 8
Q = 512          # queries per core
P = 128
DT = D // P      # 16 d tiles
KT = T // P      # 16 key tiles
RQT = RQ // P    # 4
RKT = RKV // P   # 2
HQT = HQ         # 16 q-head tiles (DH=128)
HKVT = HKV       # 4 kv-head tiles
FT = DFF // P    # 64 dff tiles
GROUP = HQ // HKV
NDIAG = Q // P   # 4 diagonal key tiles

F32 = mybir.dt.float32
BF16 = mybir.dt.bfloat16
FP8 = mybir.dt.float8e4
BF = ml_dtypes.bfloat16
F8 = ml_dtypes.float8_e4m3

# fp8 power-of-2 scales: weights 2^SW (lifts sigma~0.02 out of subnormals),
# activations 2^SH / 2^SG keep magnitudes ~16-32 << 240 (TRN e4m3 max).
SW = 9
SH = 4
SG = 4
SV = 6        # V / normalized-O fp8 scale (attention values ~0.3 rms)
SX2 = 15      # residual stream carried as 2^15*x2 so fp8 Wo psum drains 1-op
SWI = False   # DoubleRowSwInterleave weight layout (contiguous ldweights)

_CACHE = {}


def _pm():
    return (mybir.MatmulPerfMode.DoubleRowSwInterleave if SWI
            else mybir.MatmulPerfMode.DoubleRow)


def _build_nc():
    nc = bacc.Bacc("TRN2", debug=False, num_devices=NCORES)
    ap = {}
    def din(name, shape, dt=BF16):
        ap[name] = nc.dram_tensor(name, list(shape), dt, kind="ExternalInput").ap()
    din("xT", [D, T], BF16)
    din("expm", [NDIAG * P, 2, Q], BF16)   # exp(mask)^T, diagonal key tiles only
    din("bm", [P, KT], F32)                # 0 visible / -50000 fully-masked per key tile
    din("q1p", [RQT, P, DT, P])
    din("q2p", [HQT, P, RQT, P])
    din("k1p", [RKT, P, DT, P])
    din("k2p", [HKVT, P, RKT, P])
    din("v1p", [RKT, P, DT, P])
    din("v2n", [RKV, HKV * DH])
    din("woh", [HQT // 2, P, DT, 2, P], FP8)   # per-head-pair Wo, 2^SW-scaled
    din("uap", [FT, P, DT, P])
    din("ubp", [FT, P, DT, P], FP8)        # sigmoid-damped path affords fp8
    din("wdp", [DT, P, FT, P])
    outT = nc.dram_tensor("outT", [D, Q], F32, kind="ExternalOutput").ap()

    AL = mybir.AluOpType
    AF = mybir.ActivationFunctionType

    with tile.TileContext(nc) as tc, ExitStack() as ctx:
        const = ctx.enter_context(tc.tile_pool(name="const", bufs=1))
        dram = ctx.enter_context(tc.tile_pool(name="drsc", bufs=1, space="DRAM"))

        ones = const.tile([P, 1], BF16)
        nc.vector.memset(ones, 1.0)
        ones8 = const.tile([P, 2, 16], FP8)   # DoubleRow ones (16B k-pair step)
        nc.vector.memset(ones8, 1.0)
        onesc = const.tile([1, P], F32)
        nc.vector.memset(onesc, 1.0)
        onesb = const.tile([1, P], BF16)
        nc.vector.memset(onesb, 1.0)
        bm = const.tile([P, KT], F32)
        nc.scalar.dma_start(out=bm, in_=ap["bm"])

        x2pool = ctx.enter_context(tc.tile_pool(name="x2", bufs=1))
        h2pool = ctx.enter_context(tc.tile_pool(name="h2", bufs=1))

        phkv = ExitStack()
        kvq = phkv.enter_context(tc.tile_pool(name="kvq", bufs=1))
        phh = ExitStack()
        hpool = phh.enter_context(tc.tile_pool(name="h", bufs=1))
        psa_st = ExitStack()
        psA = psa_st.enter_context(tc.tile_pool(name="psA", bufs=4, space="PSUM"))

        # =============== Phase 1: load bf16 x; r1 stats ===============
        # rmsnorm scaling commutes through the linear projections: matmuls run
        # on raw bf16 x, r1 is applied per-partition on V / via Exp scale on K
        # / as a token-wide broadcast on the Q path.
        wp2 = ExitStack()
        wpool2 = wp2.enter_context(tc.tile_pool(name="w2", bufs=3))

        ph1 = ExitStack()
        st1 = ph1.enter_context(tc.tile_pool(name="st1", bufs=1))
        ssqp = ph1.enter_context(tc.tile_pool(name="ssqp", bufs=1, space="PSUM"))
        ph1sq = ExitStack()
        sqpool = ph1sq.enter_context(tc.tile_pool(name="sq", bufs=3))

        # x tiles stream first (they gate the first squares/matmuls); the
        # small-weight prefetches ride the idle gpsimd queue.
        hT = []
        for i in range(DT):
            xb = hpool.tile([P, T], BF16, name=f"xb{i}", tag=f"xb{i}")
            nc.sync.dma_start(out=xb, in_=ap["xT"][i * P:(i + 1) * P, :])
            hT.append(xb)
            if i == 3:
                w_k1, w_v1 = [], []
                for rt in range(RKT):
                    w = wpool2.tile([P, DT, P], BF16, name="wk1", tag="w16")
                    nc.gpsimd.dma_start(out=w, in_=ap["k1p"][rt])
                    w_k1.append(w)
                for rt in range(RKT):
                    w = wpool2.tile([P, DT, P], BF16, name="wv1", tag="w16")
                    nc.gpsimd.dma_start(out=w, in_=ap["v1p"][rt])
                    w_v1.append(w)

        ssq = [ssqp.tile([1, 512], F32, name=f"ssq{c}", tag=f"ssq{c}") for c in range(4)]
        for i in range(DT):
            sq = sqpool.tile([P, T], BF16, name="sq", tag="sq")
            nc.scalar.square(sq, hT[i])
            for c in range(4):
                nc.tensor.matmul(ssq[c], lhsT=ones, rhs=sq[:, c * 512:(c + 1) * 512],
                                 start=(i == 0), stop=(i == DT - 1))
        nrow = st1.tile([1, T], F32)
        for c in range(4):
            nc.scalar.activation(nrow[:, c * 512:(c + 1) * 512], ssq[c],
                                 AF.Sqrt, scale=1.0 / D)
        ph1sq.close()

        # =============== Phase 2: K/V/Q projections ===============
        # B1/KT matmuls do not need r1, so they are emitted first and the
        # r1 reciprocal chain (DRAM round-trip) hides beneath them.
        ph2 = ExitStack()
        bpool = ph2.enter_context(tc.tile_pool(name="b1", bufs=1))
        wq2pool = ph2.enter_context(tc.tile_pool(name="wq2p", bufs=3))

        B1 = {}
        for nm, wlist in (("k", w_k1), ("v", w_v1)):
            for rt in range(RKT):
                w = wlist[rt]
                bt = bpool.tile([P, T], BF16, name=f"B1{nm}{rt}", tag=f"B1{nm}{rt}")
                for c in range(4):
                    pst = psA.tile([P, 512], F32, name="ps", tag="ps")
                    for i in range(DT):
                        nc.tensor.matmul(pst, lhsT=w[:, i, :],
                                         rhs=hT[i][:, c * 512:(c + 1) * 512],
                                         start=(i == 0), stop=(i == DT - 1))
                    if c % 2 == 0:
                        nc.scalar.activation(bt[:, c * 512:(c + 1) * 512], pst,
                                             AF.Copy)
                    else:
                        nc.vector.tensor_copy(out=bt[:, c * 512:(c + 1) * 512],
                                              in_=pst)
                B1[(nm, rt)] = bt

        v2sb = kvq.tile([P, RKT, HKV * DH], BF16, name="v2", tag="v2")
        nc.scalar.dma_start(out=v2sb, in_=ap["v2n"].rearrange("(kt p) n -> p kt n", p=P))

        KTs = []
        for hd in range(HKVT):
            w = wq2pool.tile([P, RKT, P], BF16, name="wk2", tag="wk2")
            nc.scalar.dma_start(out=w, in_=ap["k2p"][hd])
            kt_sb = kvq.tile([P, T], BF16, name=f"KT{hd}", tag=f"KT{hd}")
            for c in range(4):
                pst = psA.tile([P, 512], F32, name="ps", tag="ps")
                for rt in range(RKT):
                    nc.tensor.matmul(pst, lhsT=w[:, rt, :],
                                     rhs=B1[("k", rt)][:, c * 512:(c + 1) * 512],
                                     start=(rt == 0), stop=(rt == RKT - 1))
                if c % 2 == 0:
                    nc.scalar.activation(kt_sb[:, c * 512:(c + 1) * 512], pst,
                                         AF.Copy)
                else:
                    nc.vector.tensor_copy(out=kt_sb[:, c * 512:(c + 1) * 512],
                                          in_=pst)
            KTs.append(kt_sb)
            if hd == 0:
                # r1 chain: reciprocal on 128 partitions via DRAM bounce; keep
                # r1p [P, T//P] (token t at [t % P, t // P]). Emitted here so
                # the PE chews KT matmuls while the bounce is in flight.
                nd = dram.tile([1, T], F32, name="r1nd", tag="r1nd")
                nc.gpsimd.dma_start(out=nd, in_=nrow)
                np_sb = st1.tile([P, KT], F32, name="np_sb", tag="np_sb")
                nc.gpsimd.dma_start(out=np_sb, in_=nd[0].rearrange("(t p) -> p t", p=P))
                nc.vector.tensor_scalar_add(np_sb, np_sb, EPS)
                r1p = const.tile([P, KT], F32)
                nc.vector.reciprocal(r1p, np_sb)
                r1row = st1.tile([1, Q], F32)
                nc.vector.tensor_scalar_add(r1row, nrow[:, 0:Q], EPS)
                nc.vector.reciprocal(r1row, r1row)

        # r512: per-query 1/(n+eps) broadcast across partitions via K=1 matmul
        rps = psA.tile([P, 512], F32, name="ps", tag="ps")
        nc.tensor.matmul(rps, lhsT=onesc, rhs=r1row, start=True, stop=True)
        r512 = const.tile([P, Q], F32)
        nc.vector.tensor_copy(out=r512, in_=rps)

        # V in fp8 (scale 2^SV), keys on partitions, kt-major so DoubleRow PV
        # can take kt-pairs; r1 (key rmsnorm) folded into the scale.
        r1pv = const.tile([P, KT], F32)
        nc.vector.tensor_scalar_mul(r1pv, r1p, float(2.0 ** SV))
        vq = kvq.tile([P, KT, HKV * DH], FP8, name="vq", tag="vq")
        for t in range(KT):
            pst = psA.tile([P, 512], F32, name="ps", tag="ps")
            for rt in range(RKT):
                nc.tensor.matmul(pst, lhsT=B1[("v", rt)][:, t * P:(t + 1) * P],
                                 rhs=v2sb[:, rt, :],
                                 start=(rt == 0), stop=(rt == RKT - 1))
            nc.vector.tensor_scalar_mul(vq[:, t, :], pst, r1pv[:, t:t + 1])

        A1 = []
        for rt in range(RQT):
            w = wpool2.tile([P, DT, P], BF16, name="wq1", tag="w16")
            nc.scalar.dma_start(out=w, in_=ap["q1p"][rt])
            pst = psA.tile([P, 512], F32, name="ps", tag="ps")
            for i in range(DT):
                nc.tensor.matmul(pst, lhsT=w[:, i, :], rhs=hT[i][:, 0:Q],
                                 start=(i == 0), stop=(i == DT - 1))
            a = bpool.tile([P, Q], BF16, name=f"A1{rt}", tag=f"A1{rt}")
            nc.vector.tensor_tensor(a, pst, r512, AL.mult)
            A1.append(a)
        QTs = []
        for hd in range(HQT):
            w = wq2pool.tile([P, RQT, P], BF16, name="wq2", tag="wq2")
            nc.scalar.dma_start(out=w, in_=ap["q2p"][hd])
            pst = psA.tile([P, 512], F32, name="ps", tag="ps")
            for rt in range(RQT):
                nc.tensor.matmul(pst, lhsT=w[:, rt, :], rhs=A1[rt],
                                 start=(rt == 0), stop=(rt == RQT - 1))
            qt = kvq.tile([P, Q], BF16, name=f"QT{hd}", tag=f"QT{hd}")
            if hd % 2 == 0:
                nc.scalar.activation(qt, pst, AF.Copy)
            else:
                nc.vector.tensor_copy(out=qt, in_=pst)
            QTs.append(qt)

        # residual accumulator (bf16), seeded with x; Wo partial sums add in
        # during attention, and the MLP reads/adds it at the end.
        # residual accumulator carried as 2^SX2 * x2 (bf16 exponent absorbs
        # it) so the 2^(SW+SV)-scaled fp8 Wo psum adds in with no extra op
        x2a = x2pool.tile([P, DT, Q], BF16, name="x2a", tag="x2a")
        for dm in range(DT):
            nc.vector.tensor_scalar_mul(x2a[:, dm, :], hT[dm][:, 0:Q],
                                        float(2.0 ** SX2))

        ph2.close()
        ph1.close()
        wp2.close()
        psa_st.close()
        phh.close()

        # =============== Phase 3: attention + Wo, fused ===============
        ph3 = ExitStack()
        mpool = ph3.enter_context(tc.tile_pool(name="mask", bufs=1))
        ptp = ph3.enter_context(tc.tile_pool(name="ptp", bufs=7))
        ptdp = ph3.enter_context(tc.tile_pool(name="ptdp", bufs=8))
        otp = ph3.enter_context(tc.tile_pool(name="otp", bufs=3))
        svp = ph3.enter_context(tc.tile_pool(name="svp", bufs=3))
        wohp = ph3.enter_context(tc.tile_pool(name="wohp", bufs=6))
        wops = ph3.enter_context(tc.tile_pool(name="wops", bufs=1, space="PSUM"))
        ph3a = ExitStack()
        plp = ph3a.enter_context(tc.tile_pool(name="plp", bufs=2, space="PSUM"))
        pop = ph3a.enter_context(tc.tile_pool(name="pop", bufs=1, space="PSUM"))

        expm_sb = []
        for kt in range(NDIAG):
            et = mpool.tile([P, 2, Q], BF16, name=f"em{kt}", tag=f"em{kt}")
            nc.scalar.dma_start(out=et, in_=ap["expm"][kt * P:(kt + 1) * P])
            expm_sb.append(et)

        # Software-pipelined emission: head-pair hp's normalize+Wo matmuls are
        # emitted one-by-one as gap FILLERS inside hp+1's kt loop, so the
        # in-order tensor queue never stalls (a stalled PE drops out of its
        # high p-state and runs matmuls at half clock). PV for kt is emitted
        # one iteration late so exp(kt) is always done when the queue reaches
        # it. Diagonal (partially-masked) key tiles are emitted FIRST (their
        # exp output needs a slow gpsimd expm-multiply) with their acc/PV
        # consumers LAST.
        nondiag = list(range(NDIAG, KT))

        DR = mybir.MatmulPerfMode.DoubleRow

        def wo_fillers(sts):
            """Per-tensor-queue-slot closures for 1-2 head pairs' normalize +
            Wo. Both pairs' contributions accumulate in the same psum tile so
            each x2 drain covers two head pairs."""
            for st in sts:
                def bcast(st=st):
                    wps2 = wops.tile([P, 2, Q], F32, name="wo", tag="wo")
                    for z in (0, 1):
                        nc.tensor.matmul(wps2[:, z, :], lhsT=onesb,
                                         rhs=st["sinv"][0:1, z * Q:(z + 1) * Q],
                                         start=True, stop=True)
                    # otn = 2^SV * O/S in fp8 (po carries 2^(SV+SP), sinv 2^-SP)
                    otn = otp.tile([P, 2, Q], FP8, name="otn", tag="otn")
                    nc.vector.tensor_tensor(otn, wps2, st["otr"], AL.mult)
                    st["otn"] = otn
                yield bcast
            for dp in range(DT // 2):
                def pair(dp=dp):
                    wps3 = wops.tile([P, 2, Q], F32, name="wo", tag="wo")
                    n = len(sts)
                    for si, st in enumerate(sts):
                        otn = st["otn"]
                        for z in (0, 1):
                            dm = 2 * dp + z
                            nc.tensor.matmul(wps3[:, z, :],
                                             lhsT=st["wh"][:, dm, :, :],
                                             rhs=otn,
                                             start=(si == 0), stop=(si == n - 1),
                                             perf_mode=DR)
                    nc.vector.tensor_tensor(x2a[:, 2 * dp:2 * dp + 2, :],
                                            x2a[:, 2 * dp:2 * dp + 2, :], wps3,
                                            AL.add)
                yield pair

        def s_block(st):
            """Deferred 1/S for a finished head pair. S itself was summed on
            the PE (ones^T pt into a psum lane); here just drain it and take
            a single-lane reciprocal (DVE, [1,1024]) -- no DRAM bounce."""
            def emit(st=st):
                spl = st.pop("spl")
                s_sb = svp.tile([1, 2 * Q], F32, name="ssb", tag="ssb")
                nc.scalar.activation(s_sb,
                                     spl[0:1, :, :].rearrange("o a q -> o (a q)"),
                                     AF.Copy)
                rp = svp.tile([1, 2 * Q], F32, name="rp", tag="rp")
                nc.vector.reciprocal(rp, s_sb)
                sinv_row = svp.tile([1, 2 * Q], BF16, name="sinvr", tag="sinvr")
                nc.vector.tensor_copy(out=sinv_row, in_=rp)
                st["sinv"] = sinv_row
            yield emit

        import itertools as _it
        tails = []
        fill = iter(())
        for hp in range(HQ // 2):
            h0, h1 = 2 * hp, 2 * hp + 1
            hk = h0 // GROUP
            wh = wohp.tile([P, DT, 2, P], FP8, name="wh", tag="wh")
            nc.sync.dma_start(out=wh, in_=ap["woh"][hp])

            gens = []
            if tails:
                gens.append(s_block(tails[-1]))
            if len(tails) >= 2:
                gens.append(wo_fillers([tails.pop(0)]))
            fill = _it.chain(*gens)

            po = pop.tile([P, 2, Q], F32, name="po", tag="po")

            def qk(kt, tag, bufs_pool):
                pl = plp.tile([P, 2, Q], F32, name="plp", tag="plp")
                for z in (0, 1):
                    nc.tensor.matmul(pl[:, z, :],
                                     lhsT=KTs[hk][:, kt * P:(kt + 1) * P],
                                     rhs=QTs[(h0, h1)[z]], start=True, stop=True)
                pt = bufs_pool.tile([P, 2, Q], BF16, name="pt", tag=tag)
                nc.scalar.activation(pt, pl, AF.Exp, scale=r1p[:, kt:kt + 1],
                                     bias=bm[:, kt:kt + 1])
                return pt

            def filler():
                f = next(fill, None)
                if f is not None:
                    f()

            ptd = []
            for kt in range(NDIAG):
                pt = qk(kt, "ptd", ptdp)
                filler()
                nc.gpsimd.tensor_tensor(pt, pt, expm_sb[kt], AL.mult)
                ptd.append(pt)
            # nondiag key tiles in PAIRS: exp straight to fp8, PV as one
            # DoubleRow matmul per (pair, head) contracting 256 keys. PV for
            # pair i is emitted during pair i+1 so exp is always done.
            NPAIR = (KT - NDIAG) // 2
            prev = None
            ptq = []
            for ipr in range(NPAIR):
                kt0 = NDIAG + 2 * ipr
                ptt = ptp.tile([P, 2, 2, Q], FP8, name="pt", tag="pt")
                ptq.append(ptt)
                for j in (0, 1):
                    kt = kt0 + j
                    pl = plp.tile([P, 2, Q], F32, name="plp", tag="plp")
                    for z in (0, 1):
                        nc.tensor.matmul(pl[:, z, :],
                                         lhsT=KTs[hk][:, kt * P:(kt + 1) * P],
                                         rhs=QTs[(h0, h1)[z]],
                                         start=True, stop=True)
                    nc.scalar.activation(ptt[:, j, :, :], pl, AF.Exp,
                                         scale=r1p[:, kt:kt + 1],
                                         bias=bm[:, kt:kt + 1])
                    filler()
                if prev is not None:
                    pptt, pkt0 = prev
                    for z in (0, 1):
                        nc.tensor.matmul(po[:, z, :],
                                         lhsT=vq[:, pkt0:pkt0 + 2,
                                                 hk * DH:(hk + 1) * DH],
                                         rhs=pptt[:, :, z, :],
                                         start=(ipr == 1), stop=False,
                                         perf_mode=DR)
                prev = (ptt, kt0)
            pptt, pkt0 = prev
            for z in (0, 1):
                nc.tensor.matmul(po[:, z, :],
                                 lhsT=vq[:, pkt0:pkt0 + 2, hk * DH:(hk + 1) * DH],
                                 rhs=pptt[:, :, z, :], start=False, stop=False,
                                 perf_mode=DR)
            for kt in range(NDIAG):
                pt = ptd[kt]
                for z in (0, 1):
                    nc.tensor.matmul(po[:, z, :],
                                     lhsT=vq[:, kt, hk * DH:(hk + 1) * DH],
                                     rhs=pt[:, z, :],
                                     start=False, stop=(kt == NDIAG - 1))
                filler()
            # po -> bf16 on the vector engine (scalar is pacing the exps);
            # 1/S is deferred into the next pair's filler stream.
            otr = otp.tile([P, 2, Q], BF16, name="otr", tag="otr")
            nc.vector.tensor_copy(out=otr, in_=po)
            for f in fill:
                f()
            # S on the PE: ones^T pt summed over all key tiles straight into
            # one psum lane (replaces 16 DVE adds -- DVE was the attention
            # bottleneck; fp8-operand DVE ops run at half rate).
            spl = wops.tile([P, 2, Q], F32, name="wo", tag="wo")
            for z in (0, 1):
                for ipr in range(NPAIR):
                    kt0 = NDIAG + 2 * ipr
                    nc.tensor.matmul(spl[0:1, z, :], lhsT=ones8[:, :, 0:1],
                                     rhs=ptq[ipr][:, :, z, :],
                                     start=(ipr == 0), stop=False,
                                     perf_mode=DR)
                for kt in range(NDIAG):
                    nc.tensor.matmul(spl[0:1, z, :], lhsT=ones,
                                     rhs=ptd[kt][:, z, :],
                                     start=False, stop=(kt == NDIAG - 1))
            tails.append({"spl": spl, "otr": otr, "wh": wh})
        # =============== Phase 4: rmsnorm2 (inside attention scope so the
        # final head pairs' Wo work interleaves with the squares; psum comes
        # from the wops pool since all 8 banks are still reserved) ===========
        ph3a.close()
        ph4 = ExitStack()
        st2 = ph4.enter_context(tc.tile_pool(name="st2", bufs=1))
        sq2pool = ph4.enter_context(tc.tile_pool(name="sq2", bufs=3))
        ssq2p = ph4.enter_context(tc.tile_pool(name="ssq2p", bufs=1, space="PSUM"))
        r2ps_p = ph4.enter_context(tc.tile_pool(name="r2ps", bufs=1, space="PSUM"))

        next(s_block(tails[1]))()    # S chain for the last head pair
        fin0 = wo_fillers([tails[0]])
        fin1 = wo_fillers([tails[1]])
        for f in fin0:
            f()
        next(fin1)()   # bcast for the last head pair
        ssq2 = ssq2p.tile([1, Q], F32, name="ssq2", tag="ssq2")
        for dm in range(DT):
            if dm % 2 == 0:
                next(fin1, lambda: None)()
            sq2 = sq2pool.tile([P, Q], BF16, name="sq2", tag="sq2")
            nc.scalar.square(sq2, x2a[:, dm, :])
            nc.tensor.matmul(ssq2, lhsT=ones, rhs=sq2,
                             start=(dm == 0), stop=(dm == DT - 1))
        n2 = st2.tile([1, Q], F32)
        nc.scalar.activation(n2, ssq2, AF.Sqrt, scale=1.0 / D)
        # single-lane 1/(n2+eps): slower per element than a partition-spread,
        # but avoids DRAM round-trips that contend with the MLP weight stream.
        # n2 is 2^SX2-scaled (x2a is), so eps scales too and r2 carries
        # 2^-SX2, cancelling the residual scale in h2 = r2*x2a.
        nc.vector.tensor_scalar_add(n2, n2, float(EPS * 2.0 ** SX2))
        r2f = st2.tile([1, Q], F32, name="r2f", tag="r2f")
        nc.vector.reciprocal(r2f, n2)
        r2row = st2.tile([1, Q], BF16, name="r2row", tag="r2row")
        nc.vector.tensor_copy(out=r2row, in_=r2f)
        r2ps = r2ps_p.tile([P, Q], F32, name="r2b", tag="r2b")
        nc.tensor.matmul(r2ps, lhsT=onesb, rhs=r2row, start=True, stop=True)
        r2b = st2.tile([P, Q], BF16, name="r2bs", tag="r2bs")
        nc.scalar.activation(r2b, r2ps, AF.Copy)
        # second broadcast scaled 2^SH: h2q = 2^SH * h2 in fp8 (up-B path);
        # produced on the idle gpsimd engine to spare DVE at the boundary
        r2b16 = st2.tile([P, Q], BF16, name="r2b16", tag="r2b16")
        nc.scalar.activation(r2b16, r2ps, AF.Copy, scale=float(2.0 ** SH))
        h2b = h2pool.tile([P, DT, Q], BF16, name="h2b", tag="h2b")
        h2q = h2pool.tile([P, DT, Q], FP8, name="h2q", tag="h2q")
        for dm in range(DT):
            nc.vector.tensor_tensor(h2b[:, dm, :], r2b, x2a[:, dm, :], AL.mult)
            nc.gpsimd.tensor_tensor(h2q[:, dm, :], r2b16, x2a[:, dm, :], AL.mult)
        ph4.close()
        ph3.close()
        phkv.close()

        # =============== Phase 5: SwiGLU MLP + residual ===============
        ph5 = ExitStack()
        gpool = ph5.enter_context(tc.tile_pool(name="g", bufs=1))
        psW = ph5.enter_context(tc.tile_pool(name="psW", bufs=4, space="PSUM"))
        psb = ph5.enter_context(tc.tile_pool(name="psb", bufs=4, space="PSUM"))
        wpool = ph5.enter_context(tc.tile_pool(name="w5", bufs=6))
        spool = ph5.enter_context(tc.tile_pool(name="sig", bufs=3))
        wdpool = ph5.enter_context(tc.tile_pool(name="wd", bufs=4))
        opool = ph5.enter_context(tc.tile_pool(name="out", bufs=3))

        PM = _pm()
        DP = DT // 2        # 8 contraction k-pairs for the fp8 up-B path
        g = []
        for f in range(FT):
            wa = wpool.tile([P, DT, P], BF16, name="w16", tag="w16")
            nc.sync.dma_start(out=wa, in_=ap["uap"][f])
            wb = wpool.tile([P, DT, P], FP8, name="w16b", tag="w16b")
            nc.sync.dma_start(out=wb, in_=ap["ubp"][f])
            pa = psW.tile([P, 512], F32, name="ps", tag="ps")
            pb = psb.tile([P, 512], F32, name="psb", tag="psb")
            for i in range(DT):
                nc.tensor.matmul(pa, lhsT=wa[:, i, :], rhs=h2b[:, i, :],
                                 start=(i == 0), stop=(i == DT - 1))
            for i in range(DP):
                nc.tensor.matmul(pb, lhsT=wb[:, 2 * i:2 * i + 2, :],
                                 rhs=h2q[:, 2 * i:2 * i + 2, :],
                                 start=(i == 0), stop=(i == DP - 1),
                                 perf_mode=PM)
            # pb holds 2^(SW+SH)*b
            sig = spool.tile([P, Q], F32, name="sig", tag="sig")
            nc.scalar.activation(sig, pb, AF.Sigmoid,
                                 scale=float(2.0 ** -(SW + SH)))
            gt = gpool.tile([P, Q], BF16, name=f"g{f}", tag=f"g{f}")
            nc.vector.tensor_tensor(gt, pa, sig, AL.mult)
            g.append(gt)

        H = FT // 2
        for dm in range(DT):
            wd0 = wdpool.tile([P, H, P], BF16, name="wd", tag="wd")
            nc.sync.dma_start(out=wd0, in_=ap["wdp"][dm, :, 0:H, :])
            wd1 = wdpool.tile([P, H, P], BF16, name="wd", tag="wd")
            nc.sync.dma_start(out=wd1, in_=ap["wdp"][dm, :, H:FT, :])
            pst = psW.tile([P, 512], F32, name="ps", tag="ps")
            for f in range(FT):
                wd = wd0 if f < H else wd1
                nc.tensor.matmul(pst, lhsT=wd[:, f % H, :], rhs=g[f],
                                 start=(f == 0), stop=(f == FT - 1))
            xsc = opool.tile([P, Q], BF16, name="xsc", tag="xsc")
            nc.scalar.activation(xsc, x2a[:, dm, :], AF.Copy,
                                 scale=float(2.0 ** -SX2))
            ot = opool.tile([P, Q], F32, name="outt", tag="outt")
            nc.vector.tensor_tensor(ot, pst, xsc, AL.add)
            nc.sync.dma_start(out=outT[dm * P:(dm + 1) * P, :], in_=ot)
        ph5.close()

    nc.compile()
    return nc


def _pack_lhsT(w):
    """[K, M] -> [M/128, 128, K/128, 128] so that out[mt, p, kt, c] = w[kt*128+p, mt*128+c]."""
    K, M = w.shape
    kt, mt = K // P, M // P
    return np.ascontiguousarray(
        w.reshape(kt, P, mt, P).transpose(2, 1, 0, 3)).astype(BF)


def _pack8(w, lg2s=SW):
    """fp8 DoubleRow pack: like _pack_lhsT but scaled 2^lg2s and cast e4m3.
    With SWI, each kt-pair block is stored column-interleaved+reversed
    ([A127,B127,...,A0,B0]) as DoubleRowSwInterleave expects."""
    K, M = w.shape
    kt, mt = K // P, M // P
    v = np.clip(np.asarray(w, np.float32) * (2.0 ** lg2s), -240, 240)
    q = np.ascontiguousarray(v.reshape(kt, P, mt, P).transpose(2, 1, 0, 3))
    if SWI:
        r = q.reshape(mt, P, kt // 2, 2, P)[..., ::-1]
        q = np.ascontiguousarray(r.transpose(0, 1, 2, 4, 3)).reshape(
            mt, P, kt, P)
    return q.astype(F8)


def prepare_in_maps(inputs):
    """Build the 8 per-core input dicts from the full-problem input arrays."""
    x = np.asarray(inputs["x"], np.float32)
    mask = np.asarray(inputs["attn_mask"], np.float32)[0, 0]   # [T, T]
    w1 = np.asarray(inputs["norm1_w"], np.float32)[:, None]
    w2 = np.asarray(inputs["norm2_w"], np.float32)[:, None]

    shared = {
        "q1p": _pack_lhsT(w1 * np.asarray(inputs["Wq1"], np.float32)),
        "q2p": _pack_lhsT(np.asarray(inputs["Wq2"], np.float32) / math.sqrt(DH)),
        "k1p": _pack_lhsT(w1 * np.asarray(inputs["Wk1"], np.float32)),
        "k2p": _pack_lhsT(np.asarray(inputs["Wk2"], np.float32)),
        "v1p": _pack_lhsT(w1 * np.asarray(inputs["Wv1"], np.float32)),
        "v2n": np.asarray(inputs["Wv2"], np.float32).astype(BF),
        "woh": np.ascontiguousarray(
            (np.asarray(inputs["Wo"], np.float32) * (2.0 ** SW))
            .reshape(HQT // 2, 2, P, DT, P).transpose(0, 2, 3, 1, 4)).astype(F8),
        "uap": _pack_lhsT(w2 * np.asarray(inputs["W_upA"], np.float32)),
        "ubp": _pack8(w2 * np.asarray(inputs["W_upB"], np.float32)),
        "wdp": _pack_lhsT(np.asarray(inputs["W_down"], np.float32)),
    }

    in_maps = []
    for c in range(NCORES):
        b, j = c // 4, c % 4
        xp = np.roll(x[b], -Q * j, axis=0)                     # [T, D]
        xbT = np.ascontiguousarray(xp.T).astype(BF)            # [D, T] bf16
        mq = np.roll(mask[Q * j:Q * (j + 1), :], -Q * j, axis=1)   # [Q, T]
        e1 = np.exp(mq[:, 0:Q]).T.astype(np.float32)           # [Q(keys), Q] diag block
        expm = np.ascontiguousarray(
            np.stack([e1, e1], axis=1)).astype(BF)             # [512, 2, 512]
        # key tile kt holds tokens (kt*128 + 512j) mod 2048: tiles >= 16-4j
        # wrapped to past tokens (visible); 4 <= kt < 16-4j are future (masked).
        # -ln2 halves every exp output (fp8 headroom); S sums the same halved
        # values so the normalization cancels the factor exactly.
        bm = np.full((P, KT), -math.log(2.0), np.float32)
        for kt in range(NDIAG, KT):
            if kt < KT - 4 * j:
                bm[:, kt] = -50000.0
        m = dict(shared)
        m["xT"] = xbT
        m["expm"] = expm
        m["bm"] = bm
        in_maps.append(m)
    return in_maps


def kernel(x, attn_mask, norm1_w, norm2_w, Wq1, Wq2, Wk1, Wk2, Wv1, Wv2, Wo,
           W_upA, W_upB, W_down):
    if "nc" not in _CACHE:
        _CACHE["nc"] = _build_nc()
    nc = _CACHE["nc"]

    in_maps = prepare_in_maps(dict(
        x=x, attn_mask=attn_mask, norm1_w=norm1_w, norm2_w=norm2_w,
        Wq1=Wq1, Wq2=Wq2, Wk1=Wk1, Wk2=Wk2, Wv1=Wv1, Wv2=Wv2, Wo=Wo,
        W_upA=W_upA, W_upB=W_upB, W_down=W_down))

    res = run_bass_kernel_spmd(nc, in_maps, core_ids=list(range(NCORES)))
    _CACHE["last_result"] = res

    out = np.empty((B, T, D), np.float32)
    for c in range(NCORES):
        b, j = c // 4, c % 4
        out[b, Q * j:Q * (j + 1), :] = res.results[c]["outT"].T
    return out



# revision 60
# speedup vs baseline: 1.0868x; 1.0590x over previous
"""Trainium2 Bass kernel for a dense transformer block (MLA attention + SwiGLU MLP).

Problem: B=2, T=2048, D=2048, HQ=16, HKV=4, DH=128, RQ=512, RKV=256, DFF=8192.

Sharding: sequence-parallel over 8 cores — core c owns 512 query tokens
(batch c//4, block c%4). K/V projection for the core's batch is replicated
(cheap) so NO collectives are needed. Per-core inputs are prepared on the
host: token order is rotated so the core's queries are always tokens [0:512)
(identical SPMD program on every core), and the attention-mask columns are
rotated identically so softmax over keys is order-invariant.

Layout (everything transposed, [feature, token]):
  x is uploaded bf16; r1 = 1/(sqrt(mean x^2) + eps) from on-device squares.
  B1k = Wk1^T xT ; KT = Wk2^T B1k          ([512, 2048] bf16)
  B1v = Wv1^T xT ; Vn = (B1v^T Wv2) * r1   ([2048, 512] bf16, natural)
  A1 = (Wq1^T xT[:, :512]) * r1q ; QT = (Wq2/sqrt(DH))^T A1
  attention per head pair hp, with Wo fused in:
    L^T[k,q] = KT^T QT (psum) ; P = exp(L^T * r1k + biasmask) [* expm diag]
    acc += P (vector, bf16)   ; O^T += Vn^T P (psum accum)
    S = ones^T acc (2 matmuls); 1/S via a DRAM-bounce partition spread;
    Sinv broadcast via K=1 matmul into psum; OT = O^T * Sinv;
    x2acc[dm-pair] += Wo_pair^T OT  (psum drain via vector add)
  masking: fully-masked key tiles get bias=-50000 in the exp (per-core data);
  only the 4 diagonal tiles multiply by an explicit exp(mask) tensor (on
  gpsimd, consumed only at the section end so it never paces the loop).
  Scheduling: each head pair's normalize+Wo matmuls are deferred TWO head
  pairs and emitted one-per-kt as gap fillers inside the later pair's kt
  loop — the in-order tensor queue then never stalls on the 1/S round-trip,
  which keeps the PE in its high p-state (a stalled PE drops to half clock).
  The final two pairs' Wo interleaves with the rmsnorm2 squares.
  rmsnorm2: single-lane 1/(n+eps) (no DRAM bounce: those DMAs contend with
  the MLP weight stream); r2 broadcast via K=1 matmul; SwiGLU MLP as before.

norm1_w/norm2_w are folded into Wq1/Wk1/Wv1 and WupA/WupB on the host;
1/sqrt(DH) is folded into Wq2.
"""
import math
import numpy as np
import ml_dtypes

import concourse.bass as bass
import concourse.mybir as mybir
import concourse.tile as tile
from concourse import bacc
from concourse.bass_utils import run_bass_kernel_spmd
from contextlib import ExitStack

B, T, D = 2, 2048, 2048
HQ, HKV, DH = 16, 4, 128
RQ, RKV = 512, 256
DFF = 8192
EPS = 1e-5
NCORES = 8
Q = 512          # queries per core
P = 128
DT = D // P      # 16 d tiles
KT = T // P      # 16 key tiles
RQT = RQ // P    # 4
RKT = RKV // P   # 2
HQT = HQ         # 16 q-head tiles (DH=128)
HKVT = HKV       # 4 kv-head tiles
FT = DFF // P    # 64 dff tiles
GROUP = HQ // HKV
NDIAG = Q // P   # 4 diagonal key tiles

F32 = mybir.dt.float32
BF16 = mybir.dt.bfloat16
FP8 = mybir.dt.float8e4
BF = ml_dtypes.bfloat16
F8 = ml_dtypes.float8_e4m3

# fp8 power-of-2 scales: weights 2^SW (lifts sigma~0.02 out of subnormals),
# activations 2^SH / 2^SG keep magnitudes ~16-32 << 240 (TRN e4m3 max).
SW = 9
SH = 4
SG = 4
SV = 6        # V / normalized-O fp8 scale (attention values ~0.3 rms)
SX2 = 15      # residual stream carried as 2^15*x2 so fp8 Wo psum drains 1-op
SXQ = 4       # x fp8 scale for the projection matmuls
SB = 5        # B1 / A1 low-rank intermediate fp8 scale
SWI = False   # DoubleRowSwInterleave weight layout (contiguous ldweights)

_CACHE = {}


def _pm():
    return (mybir.MatmulPerfMode.DoubleRowSwInterleave if SWI
            else mybir.MatmulPerfMode.DoubleRow)


def _build_nc():
    nc = bacc.Bacc("TRN2", debug=False, num_devices=NCORES)
    ap = {}
    def din(name, shape, dt=BF16):
        ap[name] = nc.dram_tensor(name, list(shape), dt, kind="ExternalInput").ap()
    din("xT", [D, T], BF16)
    din("xT8", [D, T], FP8)                # 2^SXQ-scaled x for fp8 projections
    din("expm", [NDIAG * P, 2, Q], BF16)   # exp(mask)^T, diagonal key tiles only
    din("bm", [P, KT], F32)                # -ln2 visible / -50000 fully-masked
    din("q1p", [RQT, P, DT, P], FP8)
    din("q2p", [HQT, P, RQT, P], FP8)
    din("k1p", [RKT, P, DT, P], FP8)
    din("k2p", [HKVT, P, RKT, P], FP8)
    din("v1p", [RKT, P, DT, P], FP8)
    din("v2n", [RKV, HKV * DH], FP8)
    din("woh", [HQT // 2, P, DT, 2, P], FP8)   # per-head-pair Wo, 2^SW-scaled
    din("uap", [FT, P, DT, P])
    din("ubp", [FT, P, DT, P], FP8)        # sigmoid-damped path affords fp8
    din("wdp", [DT, P, FT, P])
    outT = nc.dram_tensor("outT", [D, Q], F32, kind="ExternalOutput").ap()

    AL = mybir.AluOpType
    AF = mybir.ActivationFunctionType

    with tile.TileContext(nc) as tc, ExitStack() as ctx:
        const = ctx.enter_context(tc.tile_pool(name="const", bufs=1))
        dram = ctx.enter_context(tc.tile_pool(name="drsc", bufs=1, space="DRAM"))

        ones = const.tile([P, 1], BF16)
        nc.vector.memset(ones, 1.0)
        ones8 = const.tile([P, 2, 16], FP8)   # DoubleRow ones (16B k-pair step)
        nc.vector.memset(ones8, 1.0)
        onesc = const.tile([1, P], F32)
        nc.vector.memset(onesc, 1.0)
        onesb = const.tile([1, P], BF16)
        nc.vector.memset(onesb, 1.0)
        bm = const.tile([P, KT], F32)
        nc.scalar.dma_start(out=bm, in_=ap["bm"])

        x2pool = ctx.enter_context(tc.tile_pool(name="x2", bufs=1))
        h2pool = ctx.enter_context(tc.tile_pool(name="h2", bufs=1))

        phkv = ExitStack()
        kvq = phkv.enter_context(tc.tile_pool(name="kvq", bufs=1))
        phh = ExitStack()
        xqpool = phh.enter_context(tc.tile_pool(name="xq", bufs=1))
        hpool = phh.enter_context(tc.tile_pool(name="h", bufs=4))
        psa_st = ExitStack()
        psA = psa_st.enter_context(tc.tile_pool(name="psA", bufs=4, space="PSUM"))

        # =============== Phase 1: load bf16 x; r1 stats ===============
        # rmsnorm scaling commutes through the linear projections: matmuls run
        # on raw bf16 x, r1 is applied per-partition on V / via Exp scale on K
        # / as a token-wide broadcast on the Q path.
        wp2 = ExitStack()
        wpool2 = wp2.enter_context(tc.tile_pool(name="w2", bufs=3))

        ph1 = ExitStack()
        st1 = ph1.enter_context(tc.tile_pool(name="st1", bufs=1))
        ssqp = ph1.enter_context(tc.tile_pool(name="ssqp", bufs=1, space="PSUM"))
        ph1sq = ExitStack()
        sqpool = ph1sq.enter_context(tc.tile_pool(name="sq", bufs=2))

        # residual accumulator carried as 2^SX2 * x2 (bf16 exponent absorbs
        # it) so the 2^(SW+SV)-scaled fp8 Wo psum adds in with no extra op
        x2a = x2pool.tile([P, DT, Q], BF16, name="x2a", tag="x2a")

        # x tiles stream first (they gate the first squares/matmuls); the
        # small-weight prefetches ride the idle gpsimd queue.
        # x tiles stream through a small rotating pool: each is squared for
        # the r1 stats and seeds the (2^SX2-scaled) residual, then freed --
        # the projection matmuls read the fp8 copy xq instead.
        # squares in fp8 pair-tiles (x^2 <= ~29 fits e4m3; 6% elementwise
        # rounding averages out over D=2048) -> DoubleRow ones-reduction
        ssq = [ssqp.tile([1, 512], F32, name=f"ssq{c}", tag=f"ssq{c}") for c in range(4)]
        for ip in range(DT // 2):
            xb0 = hpool.tile([P, T], BF16, name="xb", tag="xb")
            nc.sync.dma_start(out=xb0, in_=ap["xT"][(2 * ip) * P:(2 * ip + 1) * P, :])
            xb1 = hpool.tile([P, T], BF16, name="xb", tag="xb")
            nc.sync.dma_start(out=xb1, in_=ap["xT"][(2 * ip + 1) * P:(2 * ip + 2) * P, :])
            if ip == 1:
                w_k1, w_v1 = [], []
                for rt in range(RKT):
                    w = wpool2.tile([P, DT, P], FP8, name="wk1", tag="w16")
                    nc.gpsimd.dma_start(out=w, in_=ap["k1p"][rt])
                    w_k1.append(w)
                for rt in range(RKT):
                    w = wpool2.tile([P, DT, P], FP8, name="wv1", tag="w16")
                    nc.gpsimd.dma_start(out=w, in_=ap["v1p"][rt])
                    w_v1.append(w)
            sqp = sqpool.tile([P, 2, T], FP8, name="sq", tag="sq")
            nc.scalar.square(sqp[:, 0, :], xb0)
            nc.scalar.square(sqp[:, 1, :], xb1)
            nc.vector.tensor_scalar_mul(x2a[:, 2 * ip, :], xb0[:, 0:Q],
                                        float(2.0 ** SX2))
            nc.vector.tensor_scalar_mul(x2a[:, 2 * ip + 1, :], xb1[:, 0:Q],
                                        float(2.0 ** SX2))
            for c in range(4):
                nc.tensor.matmul(ssq[c], lhsT=ones8[:, :, 0:1],
                                 rhs=sqp[:, :, c * 512:(c + 1) * 512],
                                 start=(ip == 0), stop=(ip == DT // 2 - 1),
                                 perf_mode=mybir.MatmulPerfMode.DoubleRow)
        xq = xqpool.tile([P, DT, T], FP8, name="xq", tag="xq")
        nc.gpsimd.dma_start(out=xq, in_=ap["xT8"].rearrange("(dt p) t -> p dt t",
                                                            p=P))
        nrow = st1.tile([1, T], F32)
        for c in range(4):
            nc.scalar.activation(nrow[:, c * 512:(c + 1) * 512], ssq[c],
                                 AF.Sqrt, scale=1.0 / D)
        ph1sq.close()

        # =============== Phase 2: K/V/Q projections ===============
        # B1/KT matmuls do not need r1, so they are emitted first and the
        # r1 reciprocal chain (DRAM round-trip) hides beneath them.
        ph2 = ExitStack()
        bpool = ph2.enter_context(tc.tile_pool(name="b1", bufs=1))
        wq2pool = ph2.enter_context(tc.tile_pool(name="wq2p", bufs=3))

        DRW = _pm()                       # stationary = static weights
        DRD = mybir.MatmulPerfMode.DoubleRow   # stationary = device-computed
        SCB = float(2.0 ** (SB - SW - SXQ))    # psum -> 2^SB-scaled fp8
        SC14 = float(2.0 ** -(SW + SB))        # psum -> unscaled bf16
        B1 = {}
        for nm, wlist in (("k", w_k1), ("v", w_v1)):
            b8 = bpool.tile([P, RKT, T], FP8, name=f"B1{nm}", tag=f"B1{nm}")
            for rt in range(RKT):
                w = wlist[rt]
                for c in range(4):
                    pst = psA.tile([P, 512], F32, name="ps", tag="ps")
                    for i in range(DT // 2):
                        nc.tensor.matmul(pst, lhsT=w[:, 2 * i:2 * i + 2, :],
                                         rhs=xq[:, 2 * i:2 * i + 2,
                                                c * 512:(c + 1) * 512],
                                         start=(i == 0), stop=(i == DT // 2 - 1),
                                         perf_mode=DRW)
                    if c % 2 == 0:
                        nc.scalar.activation(b8[:, rt, c * 512:(c + 1) * 512],
                                             pst, AF.Copy, scale=SCB)
                    else:
                        nc.vector.tensor_scalar_mul(
                            b8[:, rt, c * 512:(c + 1) * 512], pst, SCB)
            B1[nm] = b8

        v2sb = kvq.tile([P, RKT, HKV * DH], FP8, name="v2", tag="v2")
        nc.scalar.dma_start(out=v2sb, in_=ap["v2n"].rearrange("(kt p) n -> p kt n", p=P))

        KTs = []
        for hd in range(HKVT):
            w = wq2pool.tile([P, RKT, P], FP8, name="wk2", tag="wk2")
            nc.scalar.dma_start(out=w, in_=ap["k2p"][hd])
            kt_sb = kvq.tile([P, T], BF16, name=f"KT{hd}", tag=f"KT{hd}")
            for c in range(4):
                pst = psA.tile([P, 512], F32, name="ps", tag="ps")
                nc.tensor.matmul(pst, lhsT=w,
                                 rhs=B1["k"][:, :, c * 512:(c + 1) * 512],
                                 start=True, stop=True, perf_mode=DRW)
                if c % 2 == 0:
                    nc.scalar.activation(kt_sb[:, c * 512:(c + 1) * 512], pst,
                                         AF.Copy, scale=SC14)
                else:
                    nc.vector.tensor_scalar_mul(
                        kt_sb[:, c * 512:(c + 1) * 512], pst, SC14)
            KTs.append(kt_sb)
            if hd == 0:
                # r1 chain: reciprocal on 128 partitions via DRAM bounce; keep
                # r1p [P, T//P] (token t at [t % P, t // P]). Emitted here so
                # the PE chews KT matmuls while the bounce is in flight.
                nd = dram.tile([1, T], F32, name="r1nd", tag="r1nd")
                nc.gpsimd.dma_start(out=nd, in_=nrow)
                np_sb = st1.tile([P, KT], F32, name="np_sb", tag="np_sb")
                nc.gpsimd.dma_start(out=np_sb, in_=nd[0].rearrange("(t p) -> p t", p=P))
                nc.vector.tensor_scalar_add(np_sb, np_sb, EPS)
                r1p = const.tile([P, KT], F32)
                nc.vector.reciprocal(r1p, np_sb)
                r1row = st1.tile([1, Q], F32)
                nc.vector.tensor_scalar_add(r1row, nrow[:, 0:Q], EPS)
                nc.vector.reciprocal(r1row, r1row)

        # r512s: per-query 1/(n+eps) broadcast across partitions via K=1
        # matmul, pre-multiplied by the fp8 A1 drain scale 2^(SB-SW-SXQ)
        rps = psA.tile([P, 512], F32, name="ps", tag="ps")
        nc.tensor.matmul(rps, lhsT=onesc, rhs=r1row, start=True, stop=True)
        r512s = const.tile([P, Q], F32)
        nc.vector.tensor_scalar_mul(r512s, rps, float(2.0 ** (SB - SW - SXQ)))

        # V in fp8 (scale 2^SV), keys on partitions, kt-major so DoubleRow PV
        # can take kt-pairs; r1 (key rmsnorm) and the 2^-(SW+SB) psum unscale
        # folded into the per-token multiplier.
        r1pv = const.tile([P, KT], F32)
        nc.vector.tensor_scalar_mul(r1pv, r1p, float(2.0 ** (SV - SW - SB)))
        vq = kvq.tile([P, KT, HKV * DH], FP8, name="vq", tag="vq")
        for t in range(KT):
            pst = psA.tile([P, 512], F32, name="ps", tag="ps")
            nc.tensor.matmul(pst, lhsT=B1["v"][:, :, t * P:(t + 1) * P],
                             rhs=v2sb, start=True, stop=True, perf_mode=DRD)
            nc.vector.tensor_scalar_mul(vq[:, t, :], pst, r1pv[:, t:t + 1])

        a8 = bpool.tile([P, RQT, Q], FP8, name="A1", tag="A1")
        for rt in range(RQT):
            w = wpool2.tile([P, DT, P], FP8, name="wq1", tag="w16")
            nc.scalar.dma_start(out=w, in_=ap["q1p"][rt])
            pst = psA.tile([P, 512], F32, name="ps", tag="ps")
            for i in range(DT // 2):
                nc.tensor.matmul(pst, lhsT=w[:, 2 * i:2 * i + 2, :],
                                 rhs=xq[:, 2 * i:2 * i + 2, 0:Q],
                                 start=(i == 0), stop=(i == DT // 2 - 1),
                                 perf_mode=DRW)
            nc.vector.tensor_tensor(a8[:, rt, :], pst, r512s, AL.mult)
        QTs = []
        for hd in range(HQT):
            w = wq2pool.tile([P, RQT, P], FP8, name="wq2", tag="wq2")
            nc.scalar.dma_start(out=w, in_=ap["q2p"][hd])
            pst = psA.tile([P, 512], F32, name="ps", tag="ps")
            for rt in range(RQT // 2):
                nc.tensor.matmul(pst, lhsT=w[:, 2 * rt:2 * rt + 2, :],
                                 rhs=a8[:, 2 * rt:2 * rt + 2, :],
                                 start=(rt == 0), stop=(rt == RQT // 2 - 1),
                                 perf_mode=DRW)
            qt = kvq.tile([P, Q], BF16, name=f"QT{hd}", tag=f"QT{hd}")
            if hd % 2 == 0:
                nc.scalar.activation(qt, pst, AF.Copy, scale=SC14)
            else:
                nc.vector.tensor_scalar_mul(qt, pst, SC14)
            QTs.append(qt)

        # residual accumulator (bf16), seeded with x; Wo partial sums add in
        # during attention, and the MLP reads/adds it at the end.
        ph2.close()
        ph1.close()
        wp2.close()
        psa_st.close()
        phh.close()

        # =============== Phase 3: attention + Wo, fused ===============
        ph3 = ExitStack()
        mpool = ph3.enter_context(tc.tile_pool(name="mask", bufs=1))
        ptp = ph3.enter_context(tc.tile_pool(name="ptp", bufs=7))
        ptdp = ph3.enter_context(tc.tile_pool(name="ptdp", bufs=8))
        otp = ph3.enter_context(tc.tile_pool(name="otp", bufs=3))
        svp = ph3.enter_context(tc.tile_pool(name="svp", bufs=3))
        wohp = ph3.enter_context(tc.tile_pool(name="wohp", bufs=6))
        wops = ph3.enter_context(tc.tile_pool(name="wops", bufs=1, space="PSUM"))
        ph3a = ExitStack()
        plp = ph3a.enter_context(tc.tile_pool(name="plp", bufs=2, space="PSUM"))
        pop = ph3a.enter_context(tc.tile_pool(name="pop", bufs=1, space="PSUM"))

        expm_sb = []
        for kt in range(NDIAG):
            et = mpool.tile([P, 2, Q], BF16, name=f"em{kt}", tag=f"em{kt}")
            nc.scalar.dma_start(out=et, in_=ap["expm"][kt * P:(kt + 1) * P])
            expm_sb.append(et)

        # Software-pipelined emission: head-pair hp's normalize+Wo matmuls are
        # emitted one-by-one as gap FILLERS inside hp+1's kt loop, so the
        # in-order tensor queue never stalls (a stalled PE drops out of its
        # high p-state and runs matmuls at half clock). PV for kt is emitted
        # one iteration late so exp(kt) is always done when the queue reaches
        # it. Diagonal (partially-masked) key tiles are emitted FIRST (their
        # exp output needs a slow gpsimd expm-multiply) with their acc/PV
        # consumers LAST.
        nondiag = list(range(NDIAG, KT))

        DR = mybir.MatmulPerfMode.DoubleRow

        def wo_fillers(sts):
            """Per-tensor-queue-slot closures for 1-2 head pairs' normalize +
            Wo. Both pairs' contributions accumulate in the same psum tile so
            each x2 drain covers two head pairs."""
            for st in sts:
                def bcast(st=st):
                    wps2 = wops.tile([P, 2, Q], F32, name="wo", tag="wo")
                    for z in (0, 1):
                        nc.tensor.matmul(wps2[:, z, :], lhsT=onesb,
                                         rhs=st["sinv"][0:1, z * Q:(z + 1) * Q],
                                         start=True, stop=True)
                    # otn = 2^SV * O/S in fp8 (po carries 2^(SV+SP), sinv 2^-SP)
                    otn = otp.tile([P, 2, Q], FP8, name="otn", tag="otn")
                    nc.vector.tensor_tensor(otn, wps2, st["otr"], AL.mult)
                    st["otn"] = otn
                yield bcast
            for dp in range(DT // 2):
                def pair(dp=dp):
                    wps3 = wops.tile([P, 2, Q], F32, name="wo", tag="wo")
                    n = len(sts)
                    for si, st in enumerate(sts):
                        otn = st["otn"]
                        for z in (0, 1):
                            dm = 2 * dp + z
                            nc.tensor.matmul(wps3[:, z, :],
                                             lhsT=st["wh"][:, dm, :, :],
                                             rhs=otn,
                                             start=(si == 0), stop=(si == n - 1),
                                             perf_mode=DR)
                    nc.vector.tensor_tensor(x2a[:, 2 * dp:2 * dp + 2, :],
                                            x2a[:, 2 * dp:2 * dp + 2, :], wps3,
                                            AL.add)
                yield pair

        def s_block(st):
            """Deferred 1/S for a finished head pair. S was summed on the PE
            (ones^T pt into a psum lane); the reciprocal runs partition-spread
            via a DRAM round-trip (a single-lane DVE reciprocal would occupy
            the DVE 6.5us and stall the Wo-drain chain; the bounce hides in
            the ~30us slack before the consumer)."""
            def emit(st=st):
                spl = st.pop("spl")
                s_sb = svp.tile([1, 2 * Q], F32, name="ssb", tag="ssb")
                nc.scalar.activation(s_sb,
                                     spl[0:1, :, :].rearrange("o a q -> o (a q)"),
                                     AF.Copy)
                sd = dram.tile([1, 2 * Q], F32, name="sd", tag="sd")
                nc.sync.dma_start(out=sd, in_=s_sb)
                sp = svp.tile([P, 2 * Q // P], F32, name="sp", tag="sp")
                nc.sync.dma_start(out=sp, in_=sd[0].rearrange("(c p) -> p c", p=P))
                rp = svp.tile([P, 2 * Q // P], F32, name="rp", tag="rp")
                nc.vector.reciprocal(rp, sp)
                rpb = svp.tile([P, 2 * Q // P], BF16, name="rpb", tag="rpb")
                nc.vector.tensor_copy(out=rpb, in_=rp)
                rd = dram.tile([2 * Q // P, P], BF16, name="rd", tag="rd")
                nc.sync.dma_start(out=rd.rearrange("c p -> p c"), in_=rpb)
                sinv_row = svp.tile([1, 2 * Q], BF16, name="sinvr", tag="sinvr")
                nc.sync.dma_start(out=sinv_row, in_=rd.rearrange("c p -> (c p)"))
                st["sinv"] = sinv_row
            yield emit

        import itertools as _it
        tails = []
        fill = iter(())
        for hp in range(HQ // 2):
            h0, h1 = 2 * hp, 2 * hp + 1
            hk = h0 // GROUP
            wh = wohp.tile([P, DT, 2, P], FP8, name="wh", tag="wh")
            nc.sync.dma_start(out=wh, in_=ap["woh"][hp])

            gens = []
            if tails:
                gens.append(s_block(tails[-1]))
            if len(tails) >= 2:
                gens.append(wo_fillers([tails.pop(0)]))
            fill = _it.chain(*gens)

            po = pop.tile([P, 2, Q], F32, name="po", tag="po")

            def qk(kt, tag, bufs_pool):
                pl = plp.tile([P, 2, Q], F32, name="plp", tag="plp")
                for z in (0, 1):
                    nc.tensor.matmul(pl[:, z, :],
                                     lhsT=KTs[hk][:, kt * P:(kt + 1) * P],
                                     rhs=QTs[(h0, h1)[z]], start=True, stop=True)
                pt = bufs_pool.tile([P, 2, Q], BF16, name="pt", tag=tag)
                nc.scalar.activation(pt, pl, AF.Exp, scale=r1p[:, kt:kt + 1],
                                     bias=bm[:, kt:kt + 1])
                return pt

            def filler():
                f = next(fill, None)
                if f is not None:
                    f()

            ptd = []
            for kt in range(NDIAG):
                pt = qk(kt, "ptd", ptdp)
                filler()
                nc.gpsimd.tensor_tensor(pt, pt, expm_sb[kt], AL.mult)
                ptd.append(pt)
            # nondiag key tiles in PAIRS: exp straight to fp8, PV as one
            # DoubleRow matmul per (pair, head) contracting 256 keys. PV for
            # pair i is emitted during pair i+1 so exp is always done.
            NPAIR = (KT - NDIAG) // 2
            prev = None
            ptq = []
            for ipr in range(NPAIR):
                kt0 = NDIAG + 2 * ipr
                ptt = ptp.tile([P, 2, 2, Q], FP8, name="pt", tag="pt")
                ptq.append(ptt)
                for j in (0, 1):
                    kt = kt0 + j
                    pl = plp.tile([P, 2, Q], F32, name="plp", tag="plp")
                    for z in (0, 1):
                        nc.tensor.matmul(pl[:, z, :],
                                         lhsT=KTs[hk][:, kt * P:(kt + 1) * P],
                                         rhs=QTs[(h0, h1)[z]],
                                         start=True, stop=True)
                    nc.scalar.activation(ptt[:, j, :, :], pl, AF.Exp,
                                         scale=r1p[:, kt:kt + 1],
                                         bias=bm[:, kt:kt + 1])
                    filler()
                if prev is not None:
                    pptt, pkt0 = prev
                    for z in (0, 1):
                        nc.tensor.matmul(po[:, z, :],
                                         lhsT=vq[:, pkt0:pkt0 + 2,
                                                 hk * DH:(hk + 1) * DH],
                                         rhs=pptt[:, :, z, :],
                                         start=(ipr == 1), stop=False,
                                         perf_mode=DR)
                prev = (ptt, kt0)
            pptt, pkt0 = prev
            for z in (0, 1):
                nc.tensor.matmul(po[:, z, :],
                                 lhsT=vq[:, pkt0:pkt0 + 2, hk * DH:(hk + 1) * DH],
                                 rhs=pptt[:, :, z, :], start=False, stop=False,
                                 perf_mode=DR)
            for kt in range(NDIAG):
                pt = ptd[kt]
                for z in (0, 1):
                    nc.tensor.matmul(po[:, z, :],
                                     lhsT=vq[:, kt, hk * DH:(hk + 1) * DH],
                                     rhs=pt[:, z, :],
                                     start=False, stop=(kt == NDIAG - 1))
                filler()
            # po -> bf16 on the vector engine (scalar is pacing the exps);
            # 1/S is deferred into the next pair's filler stream.
            otr = otp.tile([P, 2, Q], BF16, name="otr", tag="otr")
            nc.vector.tensor_copy(out=otr, in_=po)
            for f in fill:
                f()
            # S on the PE: ones^T pt summed over all key tiles straight into
            # one psum lane (replaces 16 DVE adds -- DVE was the attention
            # bottleneck; fp8-operand DVE ops run at half rate).
            spl = wops.tile([P, 2, Q], F32, name="wo", tag="wo")
            for z in (0, 1):
                for ipr in range(NPAIR):
                    kt0 = NDIAG + 2 * ipr
                    nc.tensor.matmul(spl[0:1, z, :], lhsT=ones8[:, :, 0:1],
                                     rhs=ptq[ipr][:, :, z, :],
                                     start=(ipr == 0), stop=False,
                                     perf_mode=DR)
                for kt in range(NDIAG):
                    nc.tensor.matmul(spl[0:1, z, :], lhsT=ones,
                                     rhs=ptd[kt][:, z, :],
                                     start=False, stop=(kt == NDIAG - 1))
            tails.append({"spl": spl, "otr": otr, "wh": wh})
        # =============== Phase 4: rmsnorm2 (inside attention scope so the
        # final head pairs' Wo work interleaves with the squares; psum comes
        # from the wops pool since all 8 banks are still reserved) ===========
        ph3a.close()
        ph4 = ExitStack()
        st2 = ph4.enter_context(tc.tile_pool(name="st2", bufs=1))
        sq2pool = ph4.enter_context(tc.tile_pool(name="sq2", bufs=3))
        ssq2p = ph4.enter_context(tc.tile_pool(name="ssq2p", bufs=1, space="PSUM"))
        r2ps_p = ph4.enter_context(tc.tile_pool(name="r2ps", bufs=1, space="PSUM"))

        next(s_block(tails[1]))()    # S chain for the last head pair
        fin0 = wo_fillers([tails[0]])
        fin1 = wo_fillers([tails[1]])
        for f in fin0:
            f()
        next(fin1)()   # bcast for the last head pair
        ssq2 = ssq2p.tile([1, Q], F32, name="ssq2", tag="ssq2")
        for dm in range(DT):
            if dm % 2 == 0:
                next(fin1, lambda: None)()
            sq2 = sq2pool.tile([P, Q], BF16, name="sq2", tag="sq2")
            nc.scalar.square(sq2, x2a[:, dm, :])
            nc.tensor.matmul(ssq2, lhsT=ones, rhs=sq2,
                             start=(dm == 0), stop=(dm == DT - 1))
        n2 = st2.tile([1, Q], F32)
        nc.scalar.activation(n2, ssq2, AF.Sqrt, scale=1.0 / D)
        # single-lane 1/(n2+eps): slower per element than a partition-spread,
        # but avoids DRAM round-trips that contend with the MLP weight stream.
        # n2 is 2^SX2-scaled (x2a is), so eps scales too and r2 carries
        # 2^-SX2, cancelling the residual scale in h2 = r2*x2a.
        nc.vector.tensor_scalar_add(n2, n2, float(EPS * 2.0 ** SX2))
        r2f = st2.tile([1, Q], F32, name="r2f", tag="r2f")
        nc.vector.reciprocal(r2f, n2)
        r2row = st2.tile([1, Q], BF16, name="r2row", tag="r2row")
        nc.vector.tensor_copy(out=r2row, in_=r2f)
        r2ps = r2ps_p.tile([P, Q], F32, name="r2b", tag="r2b")
        nc.tensor.matmul(r2ps, lhsT=onesb, rhs=r2row, start=True, stop=True)
        r2b = st2.tile([P, Q], BF16, name="r2bs", tag="r2bs")
        nc.scalar.activation(r2b, r2ps, AF.Copy)
        # second broadcast scaled 2^SH: h2q = 2^SH * h2 in fp8 (up-B path).
        # Both products on DVE (gpsimd takes 2.3us/tile serially and the
        # f0 up-B matmul needs ALL h2q pairs -- it stalled the MLP start).
        r2b16 = st2.tile([P, Q], BF16, name="r2b16", tag="r2b16")
        nc.scalar.activation(r2b16, r2ps, AF.Copy, scale=float(2.0 ** SH))
        h2b = h2pool.tile([P, DT, Q], BF16, name="h2b", tag="h2b")
        h2q = h2pool.tile([P, DT, Q], FP8, name="h2q", tag="h2q")
        for dm in range(DT):
            nc.vector.tensor_tensor(h2b[:, dm, :], r2b, x2a[:, dm, :], AL.mult)
            nc.vector.tensor_tensor(h2q[:, dm, :], r2b16, x2a[:, dm, :], AL.mult)
        ph4.close()
        ph3.close()
        phkv.close()

        # =============== Phase 5: SwiGLU MLP + residual ===============
        ph5 = ExitStack()
        gpool = ph5.enter_context(tc.tile_pool(name="g", bufs=1))
        psW = ph5.enter_context(tc.tile_pool(name="psW", bufs=4, space="PSUM"))
        psb = ph5.enter_context(tc.tile_pool(name="psb", bufs=4, space="PSUM"))
        wpool = ph5.enter_context(tc.tile_pool(name="w5", bufs=6))
        spool = ph5.enter_context(tc.tile_pool(name="sig", bufs=3))
        wdpool = ph5.enter_context(tc.tile_pool(name="wd", bufs=4))
        opool = ph5.enter_context(tc.tile_pool(name="out", bufs=3))

        PM = _pm()
        DP = DT // 2        # 8 contraction k-pairs for the fp8 up-B path
        g = []
        for f in range(FT):
            wa = wpool.tile([P, DT, P], BF16, name="w16", tag="w16")
            nc.sync.dma_start(out=wa, in_=ap["uap"][f])
            wb = wpool.tile([P, DT, P], FP8, name="w16b", tag="w16b")
            nc.sync.dma_start(out=wb, in_=ap["ubp"][f])
            pa = psW.tile([P, 512], F32, name="ps", tag="ps")
            pb = psb.tile([P, 512], F32, name="psb", tag="psb")
            for i in range(DT):
                nc.tensor.matmul(pa, lhsT=wa[:, i, :], rhs=h2b[:, i, :],
                                 start=(i == 0), stop=(i == DT - 1))
            for i in range(DP):
                nc.tensor.matmul(pb, lhsT=wb[:, 2 * i:2 * i + 2, :],
                                 rhs=h2q[:, 2 * i:2 * i + 2, :],
                                 start=(i == 0), stop=(i == DP - 1),
                                 perf_mode=PM)
            # pb holds 2^(SW+SH)*b
            sig = spool.tile([P, Q], F32, name="sig", tag="sig")
            nc.scalar.activation(sig, pb, AF.Sigmoid,
                                 scale=float(2.0 ** -(SW + SH)))
            gt = gpool.tile([P, Q], BF16, name=f"g{f}", tag=f"g{f}")
            nc.vector.tensor_tensor(gt, pa, sig, AL.mult)
            g.append(gt)

        H = FT // 2
        for dm in range(DT):
            wd0 = wdpool.tile([P, H, P], BF16, name="wd", tag="wd")
            nc.sync.dma_start(out=wd0, in_=ap["wdp"][dm, :, 0:H, :])
            wd1 = wdpool.tile([P, H, P], BF16, name="wd", tag="wd")
            nc.sync.dma_start(out=wd1, in_=ap["wdp"][dm, :, H:FT, :])
            pst = psW.tile([P, 512], F32, name="ps", tag="ps")
            for f in range(FT):
                wd = wd0 if f < H else wd1
                nc.tensor.matmul(pst, lhsT=wd[:, f % H, :], rhs=g[f],
                                 start=(f == 0), stop=(f == FT - 1))
            xsc = opool.tile([P, Q], BF16, name="xsc", tag="xsc")
            nc.scalar.activation(xsc, x2a[:, dm, :], AF.Copy,
                                 scale=float(2.0 ** -SX2))
            ot = opool.tile([P, Q], F32, name="outt", tag="outt")
            nc.vector.tensor_tensor(ot, pst, xsc, AL.add)
            nc.sync.dma_start(out=outT[dm * P:(dm + 1) * P, :], in_=ot)
        ph5.close()

    nc.compile()
    return nc


def _pack_lhsT(w):
    """[K, M] -> [M/128, 128, K/128, 128] so that out[mt, p, kt, c] = w[kt*128+p, mt*128+c]."""
    K, M = w.shape
    kt, mt = K // P, M // P
    return np.ascontiguousarray(
        w.reshape(kt, P, mt, P).transpose(2, 1, 0, 3)).astype(BF)


def _pack8(w, lg2s=SW):
    """fp8 DoubleRow pack: like _pack_lhsT but scaled 2^lg2s and cast e4m3.
    With SWI, each kt-pair block is stored column-interleaved+reversed
    ([A127,B127,...,A0,B0]) as DoubleRowSwInterleave expects."""
    K, M = w.shape
    kt, mt = K // P, M // P
    v = np.clip(np.asarray(w, np.float32) * (2.0 ** lg2s), -240, 240)
    q = np.ascontiguousarray(v.reshape(kt, P, mt, P).transpose(2, 1, 0, 3))
    if SWI:
        r = q.reshape(mt, P, kt // 2, 2, P)[..., ::-1]
        q = np.ascontiguousarray(r.transpose(0, 1, 2, 4, 3)).reshape(
            mt, P, kt, P)
    return q.astype(F8)


def prepare_in_maps(inputs):
    """Build the 8 per-core input dicts from the full-problem input arrays."""
    x = np.asarray(inputs["x"], np.float32)
    mask = np.asarray(inputs["attn_mask"], np.float32)[0, 0]   # [T, T]
    w1 = np.asarray(inputs["norm1_w"], np.float32)[:, None]
    w2 = np.asarray(inputs["norm2_w"], np.float32)[:, None]

    shared = {
        "q1p": _pack8(w1 * np.asarray(inputs["Wq1"], np.float32)),
        "q2p": _pack8(np.asarray(inputs["Wq2"], np.float32) / math.sqrt(DH)),
        "k1p": _pack8(w1 * np.asarray(inputs["Wk1"], np.float32)),
        "k2p": _pack8(np.asarray(inputs["Wk2"], np.float32)),
        "v1p": _pack8(w1 * np.asarray(inputs["Wv1"], np.float32)),
        "v2n": np.clip(np.asarray(inputs["Wv2"], np.float32) * (2.0 ** SW),
                       -240, 240).astype(F8),
        "woh": np.ascontiguousarray(
            (np.asarray(inputs["Wo"], np.float32) * (2.0 ** SW))
            .reshape(HQT // 2, 2, P, DT, P).transpose(0, 2, 3, 1, 4)).astype(F8),
        "uap": _pack_lhsT(w2 * np.asarray(inputs["W_upA"], np.float32)),
        "ubp": _pack8(w2 * np.asarray(inputs["W_upB"], np.float32)),
        "wdp": _pack_lhsT(np.asarray(inputs["W_down"], np.float32)),
    }

    in_maps = []
    for c in range(NCORES):
        b, j = c // 4, c % 4
        xp = np.roll(x[b], -Q * j, axis=0)                     # [T, D]
        xbT = np.ascontiguousarray(xp.T).astype(BF)            # [D, T] bf16
        mq = np.roll(mask[Q * j:Q * (j + 1), :], -Q * j, axis=1)   # [Q, T]
        e1 = np.exp(mq[:, 0:Q]).T.astype(np.float32)           # [Q(keys), Q] diag block
        expm = np.ascontiguousarray(
            np.stack([e1, e1], axis=1)).astype(BF)             # [512, 2, 512]
        # key tile kt holds tokens (kt*128 + 512j) mod 2048: tiles >= 16-4j
        # wrapped to past tokens (visible); 4 <= kt < 16-4j are future (masked).
        # -ln2 halves every exp output (fp8 headroom); S sums the same halved
        # values so the normalization cancels the factor exactly.
        bm = np.full((P, KT), -math.log(2.0), np.float32)
        for kt in range(NDIAG, KT):
            if kt < KT - 4 * j:
                bm[:, kt] = -50000.0
        m = dict(shared)
        m["xT"] = xbT
        m["xT8"] = np.clip(np.ascontiguousarray(xp.T) * (2.0 ** SXQ),
                           -240, 240).astype(F8)
        m["expm"] = expm
        m["bm"] = bm
        in_maps.append(m)
    return in_maps


def kernel(x, attn_mask, norm1_w, norm2_w, Wq1, Wq2, Wk1, Wk2, Wv1, Wv2, Wo,
           W_upA, W_upB, W_down):
    if "nc" not in _CACHE:
        _CACHE["nc"] = _build_nc()
    nc = _CACHE["nc"]

    in_maps = prepare_in_maps(dict(
        x=x, attn_mask=attn_mask, norm1_w=norm1_w, norm2_w=norm2_w,
        Wq1=Wq1, Wq2=Wq2, Wk1=Wk1, Wk2=Wk2, Wv1=Wv1, Wv2=Wv2, Wo=Wo,
        W_upA=W_upA, W_upB=W_upB, W_down=W_down))

    res = run_bass_kernel_spmd(nc, in_maps, core_ids=list(range(NCORES)))
    _CACHE["last_result"] = res

    out = np.empty((B, T, D), np.float32)
    for c in range(NCORES):
        b, j = c // 4, c % 4
        out[b, Q * j:Q * (j + 1), :] = res.results[c]["outT"].T
    return out

